# revision 15
# baseline (speedup 1.0000x reference)
"""Trainium2 Bass kernel for nn_EquivariantLayer (spectral equivariant layer).

Data-parallel over batch: 2 samples/core x 8 cores. All-bf16 pipeline:

  stage1:  psA = f^T @ [ExR^T|ExI^T|-ExR^T | Rx^T]   (one fused matmul/2ch)
  stage2:  F = Ey @ A       (c-major conv layout via tile_position packing)
  conv:    M = F (*) K elementwise (K real); i-reduction via selector matmul
  uncurl:  pure-imaginary TO_U/TO_V -> real mults by t/s tables
  synth:   per channel pair: G = B @ QF (Q-side), field = P @ G (P-side)
  fr:      direct 2x Fourier upsample fr_i = Rx @ f_i @ Cy^T
  cross:   u_a v_b - u_b v_a in bf16 on DVE/Pool, written straight into a
           per-sample output tile; SWDGE (gpsimd) DMAs cast bf16->f32 on the
           way out to HBM.

All matmul operands are bf16 (1 cycle/row on PE vs 4 for fp32); every
PSUM->SBUF copy casts f32 accumulators down to bf16. Output 16.8MB f32 per
core dominates DMA time; compute is sized to hide beneath it.
"""
import sys
import numpy as np
import ml_dtypes

if '/opt/trn_rl_repo' not in sys.path:
    sys.path.insert(0, '/opt/trn_rl_repo')

import concourse.bass as bass
from concourse import bacc
import concourse.mybir as mybir
import concourse.tile as tile
from concourse.bass import AP
from concourse.bass_utils import run_bass_kernel_spmd

F32 = mybir.dt.float32
BF16 = mybir.dt.bfloat16
NPBF16 = ml_dtypes.bfloat16
N_CORES = 8
B_PER_CORE = 2
C1, C2, N1, N2 = 8, 16, 64, 128
NCH_OUT = 128  # 8 fr + 120 cross

I_IDX, J_IDX = np.triu_indices(C2, 1)
_PAIR_IDX = {}
for _p, (_a, _b) in enumerate(zip(I_IDX, J_IDX)):
    _PAIR_IDX[(int(_a), int(_b))] = _p


# ---------------------------------------------------------------------------
# host-side constant construction
# ---------------------------------------------------------------------------

def _host_consts():
    x = np.arange(64)
    kx = np.arange(64)
    c = np.arange(32)
    y = np.arange(64)
    X = np.arange(128)
    Y = np.arange(128)

    FRs = np.where(kx <= 32, kx, kx - 64).astype(np.float64)  # signed row freq

    ExR = np.cos(2 * np.pi * np.outer(kx, x) / 64)   # [kx, x]
    ExI = -np.sin(2 * np.pi * np.outer(kx, x) / 64)
    ExF = np.concatenate([ExR.T, ExI.T, -ExR.T], axis=1)   # [x, 192]

    EyCT = np.cos(2 * np.pi * np.outer(c, y) / 64).T   # [y=64, c=32]
    EyST = np.sin(2 * np.pi * np.outer(c, y) / 64).T
    EyCT2 = np.concatenate([EyCT, EyCT], axis=0)       # [128, 32] doubled rows
    EyST2 = np.concatenate([EyST, EyST], axis=0)

    S_sel = np.zeros((128, 32))
    for im in range(4):
        S_sel[im * 32 + np.arange(32), np.arange(32)] = 1.0

    den = FRs[None, :] ** 2 + c[:, None].astype(np.float64) ** 2
    den[0, 0] = 1.0
    t_u = c[:, None] / den                           # [32, 64]
    s_v = -FRs[None, :] / den
    t_rep = np.tile(t_u, (1, 8))                     # [32, 512] (j-rep)
    s_rep = np.tile(s_v, (1, 8))
    tsg = np.concatenate([-t_rep, t_rep, -s_rep, s_rep], axis=1)  # [32, 2048]

    w_c = np.where(c == 0, 1.0, 2.0)
    s_q = 2.0 / (128.0 * 128.0)
    QRT = (s_q * w_c[None, :] * np.cos(2 * np.pi * np.outer(Y, c) / 128)).T  # [c, Y]
    QIT = (s_q * w_c[None, :] * np.sin(2 * np.pi * np.outer(Y, c) / 128)).T
    QF1 = np.concatenate([QRT, QIT], axis=1)         # [32, 256]
    QF2 = np.concatenate([-QIT, QRT], axis=1)

    PRT = np.cos(2 * np.pi * np.outer(FRs, X) / 128)   # [kx=64, X=128]
    PIT = np.sin(2 * np.pi * np.outer(FRs, X) / 128)
    PRT[32, :] = 0.0
    PIT[32, :] = 0.0
    # doubled rows so lhsT slices can match rhs base partition 0 or 64
    PRT2 = np.concatenate([PRT, PRT], axis=0)          # [128, 128]
    nPIT2 = np.concatenate([-PIT, -PIT], axis=0)

    # direct fr path: fr_i = Rx @ f_i @ Cy^T (pure 2x Fourier upsampling)
    ExRm = np.cos(2 * np.pi * np.outer(kx, x) / 64)
    ExIm = -np.sin(2 * np.pi * np.outer(kx, x) / 64)
    EyRm = np.cos(2 * np.pi * np.outer(c, y) / 64)
    EyIm = -np.sin(2 * np.pi * np.outer(c, y) / 64)
    QRm = s_q * w_c[None, :] * np.cos(2 * np.pi * np.outer(Y, c) / 128)
    QIm = s_q * w_c[None, :] * np.sin(2 * np.pi * np.outer(Y, c) / 128)
    Rx = PRT.T @ ExRm - PIT.T @ ExIm                 # [128, 64]
    Cy = QRm @ EyRm - QIm @ EyIm                     # [128, 64]
    RxT = Rx.T                                       # [x=64, X=128]
    CyT = np.concatenate([Cy.T, Cy.T], axis=0)       # [128, 128] doubled rows

    ExFT1 = np.concatenate([ExF, RxT], axis=1)       # [64, 320]

    # pack all consts into one [128, W] bf16 blob: (rows, width, offset)
    consts = dict(ExFT1=ExFT1, EyCT=EyCT2, EyST=EyST2, S_sel=S_sel, tsg=tsg,
                  QF1=QF1, QF2=QF2, PRT=PRT2, nPIT=nPIT2, CyT=CyT)
    layout = {}
    off = 0
    for name, arr in consts.items():
        layout[name] = (arr.shape[0], arr.shape[1], off)
        off += arr.shape[1]
    blob = np.zeros((128, off), dtype=NPBF16)
    for name, arr in consts.items():
        r, w, o = layout[name]
        blob[:r, o:o + w] = arr.astype(NPBF16)
    return blob, layout


def _rot90_kernel(k):
    y = np.swapaxes(k, -2, -1)
    return np.concatenate([y[..., :1], y[..., :0:-1]], axis=-1)


def _symmetric_kernel(k):
    k1 = k
    k2 = _rot90_kernel(k1)
    k3 = _rot90_kernel(k2)
    k4 = _rot90_kernel(k3)
    k5 = np.swapaxes(k1, -2, -1)
    k6 = _rot90_kernel(k5)
    k7 = _rot90_kernel(k6)
    k8 = _rot90_kernel(k7)
    return (k1 + k2 + k3 + k4 + k5 + k6 + k7 + k8) / 8.0


def _prep_k_all(kernel_np):
    """kernel [1,8,16,64,64] -> k_all [128, 2048] conv-layout packed (bf16)."""
    ksym = _symmetric_kernel(kernel_np.astype(np.float64))[0]   # [8,16,64,64]
    K = np.fft.rfft2(ksym).real                                  # [8,16,64,33]
    Kc = np.transpose(K[:, :, :, :32], (0, 1, 3, 2)).copy()      # [i,j,c,kx]
    Kc[:, :, :, 32] = 0.0                                        # kx nyquist
    k_all = np.zeros((128, 2048), dtype=NPBF16)
    for i in range(8):
        h, im = i // 4, i % 4
        for j in range(16):
            k_all[im * 32:(im + 1) * 32, j * 128 + h * 64: j * 128 + h * 64 + 64] = \
                Kc[i, j].astype(NPBF16)
    return k_all


# ---------------------------------------------------------------------------
# device program
# ---------------------------------------------------------------------------

def _bcast(ap, n, axis_pos=1):
    """Insert a zero-step broadcast dim of size n into an AP."""
    dims = list(ap.ap)
    dims.insert(axis_pos, [0, n])
    return AP(ap.tensor, ap.offset, dims)


def _view(ap, offset_elems, dims):
    """Raw AP view on the same tensor: explicit offset (elems) + [step, count]."""
    return AP(ap.tensor, ap.offset + offset_elems, dims)


class WeightedEng:
    """Deterministic weighted round-robin over engines, balancing accumulated
    cost / weight."""

    def __init__(self, engines, weights):
        self.engines = engines
        self.w = list(weights)
        self.acc = [0.0] * len(engines)

    def pick(self, cost=1.0):
        i = min(range(len(self.engines)),
                key=lambda i: (self.acc[i] + cost) / self.w[i])
        self.acc[i] += cost
        return self.engines[i]


def build_program(reps=1, ablate=()):
    nc = bacc.Bacc("TRN2", target_bir_lowering=False)
    blob, lay = _host_consts()

    f_in = nc.dram_tensor("f_in", [B_PER_CORE, C1, 64, 64], BF16, kind="ExternalInput")
    k_in = nc.dram_tensor("k_all", [128, 2048], BF16, kind="ExternalInput")
    cb_in = nc.dram_tensor("cb", list(blob.shape), BF16, kind="ExternalInput")
    # transposed output layout [b, X, ch, Y]; host returns .transpose(0,2,1,3)
    out_sh = nc.dram_tensor("out_sh", [B_PER_CORE, 128, NCH_OUT, 128], F32,
                            kind="ExternalOutput")

    import os
    wv = float(os.environ.get("KWV", "1.6"))   # DVE weight for prod/sub split
    wp = float(os.environ.get("KWP", "1.0"))   # Pool weight

    with tile.TileContext(nc) as tc:
        with (
            tc.tile_pool(name="cp", bufs=1) as cp,
            tc.tile_pool(name="fld", bufs=2) as fld,     # per-sample u/v/out
            tc.tile_pool(name="wk", bufs=2) as wk,       # small working tiles
            tc.tile_pool(name="mw", bufs=2) as mwp,      # conv wide tiles
            tc.tile_pool(name="wp", bufs=2) as wpp,      # cross product blocks
            tc.tile_pool(name="pp", bufs=1, space="PSUM") as pp,
        ):
            # ---- load constants (one blob DMA + k) ----
            CB = cp.tile(list(blob.shape), BF16, tag="CB", name="CB")
            nc.sync.dma_start(out=CB[:], in_=cb_in[:])
            k_sb = cp.tile([128, 2048], BF16, tag="k_sb", name="k_sb")
            nc.sync.dma_start(out=k_sb[:], in_=k_in[:])

            def cview(name):
                r, w, o = lay[name]
                return CB[0:r, o:o + w]

            cExFT1 = cview("ExFT1")
            cEyCT = cview("EyCT")
            cEyST = cview("EyST")
            cS_sel = cview("S_sel")
            ctsg = cview("tsg")
            cQF1 = cview("QF1")
            cQF2 = cview("QF2")
            cPRT = cview("PRT")
            cnPIT = cview("nPIT")
            cCyT = cview("CyT")

            # product/sub engine balancer (DVE faster in bf16; Pool helps)
            ps_eng = WeightedEng([nc.vector, nc.gpsimd], [wv, wp])

            st = {b: {} for b in range(B_PER_CORE)}

            def stage1(b):
                s = st[b]
                s['OUT'] = fld.tile([128, NCH_OUT * 128], BF16, tag="OUT",
                                    name="OUT")
                fsb = wk.tile([64, 512], BF16, tag="fsb", name="fsb")
                nc.sync.dma_start(
                    out=fsb[:].rearrange("x (i y) -> x i y", i=8),
                    in_=f_in[b].rearrange("i x y -> x i y"))
                A_sb = []
                for ip in range(4):
                    psA = pp.tile([128, 320], F32, tag="bankA", bufs=2, name="psA")
                    nc.tensor.matmul(psA[:], fsb[:, ip * 128:(ip + 1) * 128],
                                     cExFT1, start=True, stop=True)
                    a_t = wk.tile([128, 320], BF16, tag=f"a{ip}", name="a_t")
                    nc.scalar.copy(out=a_t[:], in_=psA[:])
                    A_sb.append(a_t)
                s['A_sb'] = A_sb

            def stage_fr(b):
                s = st[b]
                A_sb, OUT = s['A_sb'], s['OUT']
                for iph in range(4):   # 2 channels per psUf tile
                    psUf = pp.tile([128, 256], F32, tag="bankA", bufs=2, name="psUf")
                    for iloc2 in range(2):
                        i = 2 * iph + iloc2
                        ip, iloc = i // 2, i % 2
                        t1 = A_sb[ip][iloc * 64:(iloc + 1) * 64, 192:320]
                        nc.tensor.matmul(psUf[:, iloc2 * 128:(iloc2 + 1) * 128],
                                         t1, cCyT[iloc * 64:(iloc + 1) * 64, :],
                                         start=True, stop=True)
                    nc.scalar.copy(
                        out=OUT[:, iph * 256:(iph + 1) * 256],
                        in_=psUf[:])
                if 'dma' not in ablate:
                    nc.gpsimd.dma_start(
                        out=out_sh[b, :, 0:8, :],
                        in_=OUT[:, 0:1024].rearrange("x (c y) -> x c y", c=8))

            def stage2(b):
                s = st[b]
                A_sb = s['A_sb']
                psFcv = [pp.tile([128, 128], F32, tag=f"bankF{4 + h}", name="psFcv")
                         for h in range(2)]
                for i in range(8):
                    iloc = i % 2
                    ysl = slice(iloc * 64, (iloc + 1) * 64)
                    A_RI = A_sb[i // 2][ysl, 0:128]
                    A_IS = A_sb[i // 2][ysl, 64:192]
                    h, im = i // 4, i % 4
                    sl = slice(im * 32, (im + 1) * 32)
                    tp = (0, im * 32)
                    nc.tensor.matmul(psFcv[h][sl, :], cEyCT[ysl, :], A_RI,
                                     start=True, stop=False, tile_position=tp)
                    nc.tensor.matmul(psFcv[h][sl, :], cEyST[ysl, :], A_IS,
                                     start=False, stop=True, tile_position=tp)
                Fcv = wk.tile([128, 256], BF16, tag="Fcv", name="Fcv")
                for h in range(2):
                    nc.scalar.copy(
                        out=_view(Fcv[:], h * 64,
                                  [Fcv[:].ap[0], [128, 2], [1, 64]]),
                        in_=psFcv[h][:].rearrange("p (r k) -> p r k", r=2))
                s['Fcv'] = Fcv

            def stage_conv(b):
                s = st[b]
                Fcv = s['Fcv']
                Mw = []
                for RI in range(2):
                    m_t = mwp.tile([128, 2048], BF16, tag=f"mw{RI}", name="m_t")
                    nc.vector.tensor_mul(
                        m_t[:].rearrange("p (j f) -> p j f", j=16),
                        _bcast(Fcv[:, RI * 128:(RI + 1) * 128], 16),
                        k_sb[:].rearrange("p (j f) -> p j f", j=16))
                    Mw.append(m_t)

                acv_sb = wk.tile([32, 2048], BF16, tag="acv", name="acv_sb")
                for RI in range(2):
                    for jh in range(2):
                        ps_acv = pp.tile([32, 512], F32, tag="bankA", bufs=2,
                                         name="ps_acv")
                        for h in range(2):
                            rhs = _view(Mw[RI][:], jh * 1024 + h * 64,
                                        [Mw[RI][:].ap[0], [128, 8], [1, 64]])
                            nc.tensor.matmul(ps_acv[:], cS_sel, rhs,
                                             start=(h == 0), stop=(h == 1))
                        nc.scalar.copy(
                            out=acv_sb[:, (RI * 2 + jh) * 512:(RI * 2 + jh + 1) * 512],
                            in_=ps_acv[:])
                s['acv_sb'] = acv_sb

            def stage_B(b):
                s = st[b]
                acv_sb = s['acv_sb']
                # ---------------- uncurl: B = acv (*) t/s ----------------
                BuR = wk.tile([32, 1024], BF16, tag="BuR", name="BuR")
                BuI = wk.tile([32, 1024], BF16, tag="BuI", name="BuI")
                BvR = wk.tile([32, 1024], BF16, tag="BvR", name="BvR")
                BvI = wk.tile([32, 1024], BF16, tag="BvI", name="BvI")
                # acv layout: [R jh0 | R jh1 | I jh0 | I jh1] each 512
                # tsg: [-t | t | -s | s] each 512 (j-repeated, j-independent)
                for RI in range(2):
                    a_v = _view(acv_sb[:], RI * 1024,
                                [acv_sb[:].ap[0], [512, 2], [1, 512]])
                    if RI == 0:  # A_R -> imag parts (mult by +t / +s)
                        nc.vector.tensor_mul(
                            BuI[:].rearrange("p (j f) -> p j f", j=2),
                            a_v, _bcast(ctsg[:, 512:1024], 2))
                        nc.vector.tensor_mul(
                            BvI[:].rearrange("p (j f) -> p j f", j=2),
                            a_v, _bcast(ctsg[:, 1536:2048], 2))
                    else:        # A_I -> real parts (mult by -t / -s)
                        nc.vector.tensor_mul(
                            BuR[:].rearrange("p (j f) -> p j f", j=2),
                            a_v, _bcast(ctsg[:, 0:512], 2))
                        nc.vector.tensor_mul(
                            BvR[:].rearrange("p (j f) -> p j f", j=2),
                            a_v, _bcast(ctsg[:, 1024:1536], 2))
                s['B'] = (BuR, BuI, BvR, BvI)
                s['u_all'] = fld.tile([128, 2048], BF16, tag="u_all", name="u_all")
                s['v_all'] = fld.tile([128, 2048], BF16, tag="v_all", name="v_all")

            tog_ctr = [0]

            def synth_group(b, field, g2):
                """4 channels (2 cpairs) -> dest[:, g2*512:(g2+1)*512]."""
                s = st[b]
                BuR, BuI, BvR, BvI = s['B']
                BR, BI = (BuR, BuI) if field == 'u' else (BvR, BvI)
                dest = s['u_all'] if field == 'u' else s['v_all']
                tog = tog_ctr[0]
                tog_ctr[0] += 1
                psG = pp.tile([128, 512], F32, tag=f"bankF{tog % 2}",
                              name="psG")
                for sub in range(2):
                    cpair = 2 * g2 + sub
                    csl = slice(cpair * 128, (cpair + 1) * 128)
                    osl = slice(sub * 256, (sub + 1) * 256)
                    nc.tensor.matmul(psG[:, osl], BR[:, csl], cQF1,
                                     start=True, stop=False)
                    nc.tensor.matmul(psG[:, osl], BI[:, csl], cQF2,
                                     start=False, stop=True)
                G_sb = wk.tile([128, 512], BF16, tag="G_sb", name="G_sb")
                nc.scalar.copy(out=G_sb[:], in_=psG[:])
                psU = pp.tile([128, 512], F32, tag=f"bankF{2 + tog % 2}",
                              name="psU")
                for chl in range(4):
                    sub, chp = chl // 2, chl % 2
                    gr = G_sb[chp * 64:(chp + 1) * 64,
                              sub * 256:sub * 256 + 128]
                    gi = G_sb[chp * 64:(chp + 1) * 64,
                              sub * 256 + 128:sub * 256 + 256]
                    psl = slice(chp * 64, (chp + 1) * 64)
                    osl = slice(chl * 128, (chl + 1) * 128)
                    nc.tensor.matmul(psU[:, osl], cPRT[psl, :], gr,
                                     start=True, stop=False)
                    nc.tensor.matmul(psU[:, osl], cnPIT[psl, :], gi,
                                     start=False, stop=True)
                nc.scalar.copy(out=dest[:, g2 * 512:(g2 + 1) * 512],
                               in_=psU[:])

            def emit_cross_block(b, gI, gJ):
                u_all, v_all, OUT = st[b]['u_all'], st[b]['v_all'], st[b]['OUT']
                W1 = wpp.tile([128, 2048], BF16, tag="W1", name="W1")
                for ai in range(4):
                    a = 4 * gI + ai
                    in0 = _view(u_all[:], a * 128,
                                [u_all[:].ap[0], [0, 4], [1, 128]])
                    in1 = _view(v_all[:], gJ * 512,
                                [v_all[:].ap[0], [128, 4], [1, 128]])
                    out = W1[:, ai * 512:(ai + 1) * 512].rearrange(
                        "p (cb f) -> p cb f", cb=4)
                    ps_eng.pick(1.0).tensor_mul(out, in0, in1)
                if gI != gJ:
                    W2 = wpp.tile([128, 2048], BF16, tag="W2", name="W2")
                    for bjl in range(4):
                        bj = 4 * gJ + bjl
                        in0 = _view(u_all[:], bj * 128,
                                    [u_all[:].ap[0], [0, 4], [1, 128]])
                        in1 = _view(v_all[:], gI * 512,
                                    [v_all[:].ap[0], [128, 4], [1, 128]])
                        out = W2[:, bjl * 512:(bjl + 1) * 512].rearrange(
                            "p (ca f) -> p ca f", ca=4)
                        ps_eng.pick(1.0).tensor_mul(out, in0, in1)
                    for ai in range(4):
                        a = 4 * gI + ai
                        pch = 8 + _PAIR_IDX[(a, 4 * gJ)]
                        in0 = W1[:, ai * 512:(ai + 1) * 512].rearrange(
                            "p (cb f) -> p cb f", cb=4)
                        in1 = _view(W2[:], ai * 128,
                                    [W2[:].ap[0], [512, 4], [1, 128]])
                        out = _view(OUT[:], pch * 128,
                                    [OUT[:].ap[0], [128, 4], [1, 128]])
                        ps_eng.pick(1.0).tensor_sub(out, in0, in1)
                else:
                    for ai in range(3):
                        a = 4 * gI + ai
                        cnt = 3 - ai
                        pch = 8 + _PAIR_IDX[(a, a + 1)]
                        in0 = _view(W1[:], ai * 512 + (ai + 1) * 128,
                                    [W1[:].ap[0], [128, cnt], [1, 128]])
                        in1 = _view(W1[:], (ai + 1) * 512 + ai * 128,
                                    [W1[:].ap[0], [512, cnt], [1, 128]])
                        out = _view(OUT[:], pch * 128,
                                    [OUT[:].ap[0], [128, cnt], [1, 128]])
                        ps_eng.pick(cnt / 4.0).tensor_sub(out, in0, in1)

            def cross_dma(b, c0, c1):
                OUT = st[b]['OUT']
                nc.gpsimd.dma_start(
                    out=out_sh[b, :, c0:c1, :],
                    in_=OUT[:, c0 * 128:c1 * 128].rearrange(
                        "x (c y) -> x c y", c=c1 - c0))

            # ---------------- interleaved emission across samples ----------
            Bs = list(range(B_PER_CORE))
            for b in Bs:
                stage1(b)
            for b in Bs:
                stage_fr(b)
            for b in Bs:
                stage2(b)
            for b in Bs:
                stage_conv(b)
            for b in Bs:
                stage_B(b)
            for g2 in range(4):          # all v first (cross needs full v)
                for b in Bs:
                    synth_group(b, 'v', g2)
            # u groups interleaved with cross rows; DMA chunks as channel
            # ranges complete.  gI=0 covers pch 8..61, gI=1 -> ..99,
            # gI=2 -> ..121, gI=3 -> ..127.
            chunk_hi = [62, 100, 122, 128]
            chunk_lo = 8
            for gI in range(4):
                for b in Bs:
                    synth_group(b, 'u', gI)
                for gJ in range(gI, 4):
                    for b in Bs:
                        emit_cross_block(b, gI, gJ)
                if 'dma' not in ablate:
                    for b in Bs:
                        cross_dma(b, chunk_lo, chunk_hi[gI])
                chunk_lo = chunk_hi[gI]
    nc.compile()
    return nc


# ---------------------------------------------------------------------------
# entry point
# ---------------------------------------------------------------------------

_PROGRAM = {}


def _get_program(reps=1, ablate=(), **kw):
    key = (reps, tuple(sorted(ablate)), tuple(sorted(kw.items())))
    if key not in _PROGRAM:
        _PROGRAM[key] = build_program(reps, ablate=ablate, **kw)
    return _PROGRAM[key]


LAST_EXEC_NS = None
LAST_RESULT = None


def kernel(f, kernel):
    global LAST_EXEC_NS, LAST_RESULT
    f_bf = np.ascontiguousarray(np.asarray(f), dtype=np.float32).astype(NPBF16)
    k_all = _prep_k_all(np.asarray(kernel))
    blob, _ = _host_consts()
    nc = _get_program()
    in_maps = [
        {"f_in": f_bf[2 * c:2 * c + 2], "k_all": k_all, "cb": blob}
        for c in range(N_CORES)
    ]
    import os
    trace = bool(os.environ.get("KERNEL_TRACE"))
    res = run_bass_kernel_spmd(nc, in_maps, list(range(N_CORES)), trace=trace)
    LAST_RESULT = res
    if res.exec_time_ns is not None:
        LAST_EXEC_NS = res.exec_time_ns
    out = np.concatenate([res.results[c]["out_sh"] for c in range(N_CORES)], axis=0)
    # device layout is [b, X, ch, Y]; return the [b, ch, X, Y] view
    return out.transpose(0, 2, 1, 3)


# revision 18
# speedup vs baseline: 1.0726x; 1.0726x over previous
"""Trainium2 Bass kernel for nn_EquivariantLayer (spectral equivariant layer).

Data-parallel over batch: 2 samples/core x 8 cores. All-bf16 pipeline:

  stage1:  psA = f^T @ [ExR^T|ExI^T|-ExR^T | Rx^T]   (one fused matmul/2ch)
  stage2:  F = Ey @ A       (c-major conv layout via tile_position packing)
  conv:    M = F (*) K elementwise (K real); i-reduction via selector matmul
  uncurl:  pure-imaginary TO_U/TO_V -> real mults by t/s tables
  synth:   per channel pair: G = B @ QF (Q-side), field = P @ G (P-side)
  fr:      direct 2x Fourier upsample fr_i = Rx @ f_i @ Cy^T
  cross:   u_a v_b - u_b v_a in bf16 on DVE/Pool, written straight into a
           per-sample output tile; SWDGE (gpsimd) DMAs cast bf16->f32 on the
           way out to HBM.

All matmul operands are bf16 (1 cycle/row on PE vs 4 for fp32); every
PSUM->SBUF copy casts f32 accumulators down to bf16. Output 16.8MB f32 per
core dominates DMA time; compute is sized to hide beneath it.
"""
import sys
import numpy as np
import ml_dtypes

if '/opt/trn_rl_repo' not in sys.path:
    sys.path.insert(0, '/opt/trn_rl_repo')

import concourse.bass as bass
from concourse import bacc
import concourse.mybir as mybir
import concourse.tile as tile
from concourse.bass import AP
from concourse.bass_utils import run_bass_kernel_spmd

F32 = mybir.dt.float32
BF16 = mybir.dt.bfloat16
NPBF16 = ml_dtypes.bfloat16
N_CORES = 8
B_PER_CORE = 2
C1, C2, N1, N2 = 8, 16, 64, 128
NCH_OUT = 128  # 8 fr + 120 cross

I_IDX, J_IDX = np.triu_indices(C2, 1)
_PAIR_IDX = {}
for _p, (_a, _b) in enumerate(zip(I_IDX, J_IDX)):
    _PAIR_IDX[(int(_a), int(_b))] = _p


# ---------------------------------------------------------------------------
# host-side constant construction
# ---------------------------------------------------------------------------

def _host_consts():
    x = np.arange(64)
    kx = np.arange(64)
    c = np.arange(32)
    y = np.arange(64)
    X = np.arange(128)
    Y = np.arange(128)

    FRs = np.where(kx <= 32, kx, kx - 64).astype(np.float64)  # signed row freq

    ExR = np.cos(2 * np.pi * np.outer(kx, x) / 64)   # [kx, x]
    ExI = -np.sin(2 * np.pi * np.outer(kx, x) / 64)
    ExF = np.concatenate([ExR.T, ExI.T, -ExR.T], axis=1)   # [x, 192]

    EyCT = np.cos(2 * np.pi * np.outer(c, y) / 64).T   # [y=64, c=32]
    EyST = np.sin(2 * np.pi * np.outer(c, y) / 64).T
    EyCT2 = np.concatenate([EyCT, EyCT], axis=0)       # [128, 32] doubled rows
    EyST2 = np.concatenate([EyST, EyST], axis=0)

    S_sel = np.zeros((128, 32))
    for im in range(4):
        S_sel[im * 32 + np.arange(32), np.arange(32)] = 1.0

    den = FRs[None, :] ** 2 + c[:, None].astype(np.float64) ** 2
    den[0, 0] = 1.0
    t_u = c[:, None] / den                           # [32, 64]
    s_v = -FRs[None, :] / den
    t_rep = np.tile(t_u, (1, 8))                     # [32, 512] (j-rep)
    s_rep = np.tile(s_v, (1, 8))
    tsg = np.concatenate([-t_rep, t_rep, -s_rep, s_rep], axis=1)  # [32, 2048]

    w_c = np.where(c == 0, 1.0, 2.0)
    s_q = 2.0 / (128.0 * 128.0)
    QRT = (s_q * w_c[None, :] * np.cos(2 * np.pi * np.outer(Y, c) / 128)).T  # [c, Y]
    QIT = (s_q * w_c[None, :] * np.sin(2 * np.pi * np.outer(Y, c) / 128)).T
    QF1 = np.concatenate([QRT, QIT], axis=1)         # [32, 256]
    QF2 = np.concatenate([-QIT, QRT], axis=1)

    PRT = np.cos(2 * np.pi * np.outer(FRs, X) / 128)   # [kx=64, X=128]
    PIT = np.sin(2 * np.pi * np.outer(FRs, X) / 128)
    PRT[32, :] = 0.0
    PIT[32, :] = 0.0
    # doubled rows so lhsT slices can match rhs base partition 0 or 64
    PRT2 = np.concatenate([PRT, PRT], axis=0)          # [128, 128]
    nPIT2 = np.concatenate([-PIT, -PIT], axis=0)

    # direct fr path: fr_i = Rx @ f_i @ Cy^T (pure 2x Fourier upsampling)
    ExRm = np.cos(2 * np.pi * np.outer(kx, x) / 64)
    ExIm = -np.sin(2 * np.pi * np.outer(kx, x) / 64)
    EyRm = np.cos(2 * np.pi * np.outer(c, y) / 64)
    EyIm = -np.sin(2 * np.pi * np.outer(c, y) / 64)
    QRm = s_q * w_c[None, :] * np.cos(2 * np.pi * np.outer(Y, c) / 128)
    QIm = s_q * w_c[None, :] * np.sin(2 * np.pi * np.outer(Y, c) / 128)
    Rx = PRT.T @ ExRm - PIT.T @ ExIm                 # [128, 64]
    Cy = QRm @ EyRm - QIm @ EyIm                     # [128, 64]
    RxT = Rx.T                                       # [x=64, X=128]
    CyT = np.concatenate([Cy.T, Cy.T], axis=0)       # [128, 128] doubled rows

    ExFT1 = np.concatenate([ExF, RxT], axis=1)       # [64, 320]

    # pack all consts into one [128, W] bf16 blob: (rows, width, offset)
    consts = dict(ExFT1=ExFT1, EyCT=EyCT2, EyST=EyST2, S_sel=S_sel, tsg=tsg,
                  QF1=QF1, QF2=QF2, PRT=PRT2, nPIT=nPIT2, CyT=CyT)
    layout = {}
    off = 0
    for name, arr in consts.items():
        layout[name] = (arr.shape[0], arr.shape[1], off)
        off += arr.shape[1]
    blob = np.zeros((128, off), dtype=NPBF16)
    for name, arr in consts.items():
        r, w, o = layout[name]
        blob[:r, o:o + w] = arr.astype(NPBF16)
    return blob, layout


def _rot90_kernel(k):
    y = np.swapaxes(k, -2, -1)
    return np.concatenate([y[..., :1], y[..., :0:-1]], axis=-1)


def _symmetric_kernel(k):
    k1 = k
    k2 = _rot90_kernel(k1)
    k3 = _rot90_kernel(k2)
    k4 = _rot90_kernel(k3)
    k5 = np.swapaxes(k1, -2, -1)
    k6 = _rot90_kernel(k5)
    k7 = _rot90_kernel(k6)
    k8 = _rot90_kernel(k7)
    return (k1 + k2 + k3 + k4 + k5 + k6 + k7 + k8) / 8.0


def _prep_k_all(kernel_np):
    """kernel [1,8,16,64,64] -> k_all [128, 2048] conv-layout packed (bf16)."""
    ksym = _symmetric_kernel(kernel_np.astype(np.float64))[0]   # [8,16,64,64]
    K = np.fft.rfft2(ksym).real                                  # [8,16,64,33]
    Kc = np.transpose(K[:, :, :, :32], (0, 1, 3, 2)).copy()      # [i,j,c,kx]
    Kc[:, :, :, 32] = 0.0                                        # kx nyquist
    k_all = np.zeros((128, 2048), dtype=NPBF16)
    for i in range(8):
        h, im = i // 4, i % 4
        for j in range(16):
            k_all[im * 32:(im + 1) * 32, j * 128 + h * 64: j * 128 + h * 64 + 64] = \
                Kc[i, j].astype(NPBF16)
    return k_all


# ---------------------------------------------------------------------------
# device program
# ---------------------------------------------------------------------------

def _bcast(ap, n, axis_pos=1):
    """Insert a zero-step broadcast dim of size n into an AP."""
    dims = list(ap.ap)
    dims.insert(axis_pos, [0, n])
    return AP(ap.tensor, ap.offset, dims)


def _view(ap, offset_elems, dims):
    """Raw AP view on the same tensor: explicit offset (elems) + [step, count]."""
    return AP(ap.tensor, ap.offset + offset_elems, dims)


class WeightedEng:
    """Deterministic weighted round-robin over engines, balancing accumulated
    cost / weight."""

    def __init__(self, engines, weights):
        self.engines = engines
        self.w = list(weights)
        self.acc = [0.0] * len(engines)

    def pick(self, cost=1.0):
        i = min(range(len(self.engines)),
                key=lambda i: (self.acc[i] + cost) / self.w[i])
        self.acc[i] += cost
        return self.engines[i]


def build_program(reps=1, ablate=()):
    nc = bacc.Bacc("TRN2", target_bir_lowering=False)
    blob, lay = _host_consts()

    f_in = nc.dram_tensor("f_in", [B_PER_CORE, C1, 64, 64], BF16, kind="ExternalInput")
    k_in = nc.dram_tensor("k_all", [128, 2048], BF16, kind="ExternalInput")
    cb_in = nc.dram_tensor("cb", list(blob.shape), BF16, kind="ExternalInput")
    # transposed output layout [b, X, ch, Y]; host returns .transpose(0,2,1,3)
    out_sh = nc.dram_tensor("out_sh", [B_PER_CORE, 128, NCH_OUT, 128], F32,
                            kind="ExternalOutput")

    import os
    wv = float(os.environ.get("KWV", "1.6"))   # DVE weight for prod/sub split
    wp = float(os.environ.get("KWP", "1.0"))   # Pool weight

    with tile.TileContext(nc) as tc:
        with (
            tc.tile_pool(name="cp", bufs=1) as cp,
            tc.tile_pool(name="fld", bufs=2) as fld,     # per-sample u/v/out
            tc.tile_pool(name="wk", bufs=2) as wk,       # small working tiles
            tc.tile_pool(name="mw", bufs=2) as mwp,      # conv wide tiles
            tc.tile_pool(name="wp", bufs=2) as wpp,      # cross product blocks
            tc.tile_pool(name="pp", bufs=1, space="PSUM") as pp,
        ):
            # ---- load constants (one blob DMA + k) ----
            CB = cp.tile(list(blob.shape), BF16, tag="CB", name="CB")
            nc.sync.dma_start(out=CB[:], in_=cb_in[:])
            k_sb = cp.tile([128, 2048], BF16, tag="k_sb", name="k_sb")
            nc.sync.dma_start(out=k_sb[:], in_=k_in[:])

            def cview(name):
                r, w, o = lay[name]
                return CB[0:r, o:o + w]

            cExFT1 = cview("ExFT1")
            cEyCT = cview("EyCT")
            cEyST = cview("EyST")
            cS_sel = cview("S_sel")
            ctsg = cview("tsg")
            cQF1 = cview("QF1")
            cQF2 = cview("QF2")
            cPRT = cview("PRT")
            cnPIT = cview("nPIT")
            cCyT = cview("CyT")

            # product/sub engine balancer (DVE faster in bf16; Pool helps)
            ps_eng = WeightedEng([nc.vector, nc.gpsimd], [wv, wp])

            st = {b: {} for b in range(B_PER_CORE)}

            def stage1(b):
                s = st[b]
                s['OUT'] = fld.tile([128, NCH_OUT * 128], BF16, tag="OUT",
                                    name="OUT")
                fsb = wk.tile([64, 512], BF16, tag="fsb", name="fsb")
                nc.sync.dma_start(
                    out=fsb[:].rearrange("x (i y) -> x i y", i=8),
                    in_=f_in[b].rearrange("i x y -> x i y"))
                A_sb = []
                for ip in range(4):
                    psA = pp.tile([128, 320], F32, tag="bankA", bufs=2, name="psA")
                    nc.tensor.matmul(psA[:], fsb[:, ip * 128:(ip + 1) * 128],
                                     cExFT1, start=True, stop=True)
                    a_t = wk.tile([128, 320], BF16, tag=f"a{ip}", name="a_t")
                    nc.scalar.copy(out=a_t[:], in_=psA[:])
                    A_sb.append(a_t)
                s['A_sb'] = A_sb

            def stage_fr(b):
                s = st[b]
                A_sb, OUT = s['A_sb'], s['OUT']
                for iph in range(4):   # 2 channels per psUf tile
                    psUf = pp.tile([128, 256], F32, tag="bankA", bufs=2, name="psUf")
                    for iloc2 in range(2):
                        i = 2 * iph + iloc2
                        ip, iloc = i // 2, i % 2
                        t1 = A_sb[ip][iloc * 64:(iloc + 1) * 64, 192:320]
                        nc.tensor.matmul(psUf[:, iloc2 * 128:(iloc2 + 1) * 128],
                                         t1, cCyT[iloc * 64:(iloc + 1) * 64, :],
                                         start=True, stop=True)
                    nc.scalar.copy(
                        out=OUT[:, iph * 256:(iph + 1) * 256],
                        in_=psUf[:])
                if 'dma' not in ablate:
                    nc.gpsimd.dma_start(
                        out=out_sh[b, :, 0:8, :],
                        in_=OUT[:, 0:1024].rearrange("x (c y) -> x c y", c=8))

            def stage2(b):
                s = st[b]
                A_sb = s['A_sb']
                psFcv = [pp.tile([128, 128], F32, tag=f"bankF{4 + h}", name="psFcv")
                         for h in range(2)]
                for i in range(8):
                    iloc = i % 2
                    ysl = slice(iloc * 64, (iloc + 1) * 64)
                    A_RI = A_sb[i // 2][ysl, 0:128]
                    A_IS = A_sb[i // 2][ysl, 64:192]
                    h, im = i // 4, i % 4
                    sl = slice(im * 32, (im + 1) * 32)
                    tp = (0, im * 32)
                    nc.tensor.matmul(psFcv[h][sl, :], cEyCT[ysl, :], A_RI,
                                     start=True, stop=False, tile_position=tp)
                    nc.tensor.matmul(psFcv[h][sl, :], cEyST[ysl, :], A_IS,
                                     start=False, stop=True, tile_position=tp)
                Fcv = wk.tile([128, 256], BF16, tag="Fcv", name="Fcv")
                for h in range(2):
                    nc.scalar.copy(
                        out=_view(Fcv[:], h * 64,
                                  [Fcv[:].ap[0], [128, 2], [1, 64]]),
                        in_=psFcv[h][:].rearrange("p (r k) -> p r k", r=2))
                s['Fcv'] = Fcv

            def stage_conv(b):
                s = st[b]
                Fcv = s['Fcv']
                Mw = []
                for RI in range(2):
                    m_t = mwp.tile([128, 2048], BF16, tag=f"mw{RI}", name="m_t")
                    conv_eng = nc.vector if RI == 0 else nc.gpsimd
                    conv_eng.tensor_mul(
                        m_t[:].rearrange("p (j f) -> p j f", j=16),
                        _bcast(Fcv[:, RI * 128:(RI + 1) * 128], 16),
                        k_sb[:].rearrange("p (j f) -> p j f", j=16))
                    Mw.append(m_t)

                acv_sb = wk.tile([32, 2048], BF16, tag="acv", name="acv_sb")
                for RI in range(2):
                    for jh in range(2):
                        ps_acv = pp.tile([32, 512], F32, tag="bankA", bufs=2,
                                         name="ps_acv")
                        for h in range(2):
                            rhs = _view(Mw[RI][:], jh * 1024 + h * 64,
                                        [Mw[RI][:].ap[0], [128, 8], [1, 64]])
                            nc.tensor.matmul(ps_acv[:], cS_sel, rhs,
                                             start=(h == 0), stop=(h == 1))
                        nc.scalar.copy(
                            out=acv_sb[:, (RI * 2 + jh) * 512:(RI * 2 + jh + 1) * 512],
                            in_=ps_acv[:])
                s['acv_sb'] = acv_sb

            def stage_B(b):
                s = st[b]
                acv_sb = s['acv_sb']
                # ---------------- uncurl: B = acv (*) t/s ----------------
                BuR = wk.tile([32, 1024], BF16, tag="BuR", name="BuR")
                BuI = wk.tile([32, 1024], BF16, tag="BuI", name="BuI")
                BvR = wk.tile([32, 1024], BF16, tag="BvR", name="BvR")
                BvI = wk.tile([32, 1024], BF16, tag="BvI", name="BvI")
                # acv layout: [R jh0 | R jh1 | I jh0 | I jh1] each 512
                # tsg: [-t | t | -s | s] each 512 (j-repeated, j-independent)
                for RI in range(2):
                    a_v = _view(acv_sb[:], RI * 1024,
                                [acv_sb[:].ap[0], [512, 2], [1, 512]])
                    if RI == 0:  # A_R -> imag parts (mult by +t / +s)
                        nc.vector.tensor_mul(
                            BuI[:].rearrange("p (j f) -> p j f", j=2),
                            a_v, _bcast(ctsg[:, 512:1024], 2))
                        nc.vector.tensor_mul(
                            BvI[:].rearrange("p (j f) -> p j f", j=2),
                            a_v, _bcast(ctsg[:, 1536:2048], 2))
                    else:        # A_I -> real parts (mult by -t / -s)
                        nc.vector.tensor_mul(
                            BuR[:].rearrange("p (j f) -> p j f", j=2),
                            a_v, _bcast(ctsg[:, 0:512], 2))
                        nc.vector.tensor_mul(
                            BvR[:].rearrange("p (j f) -> p j f", j=2),
                            a_v, _bcast(ctsg[:, 1024:1536], 2))
                s['B'] = (BuR, BuI, BvR, BvI)
                s['u_all'] = fld.tile([128, 2048], BF16, tag="u_all", name="u_all")
                s['v_all'] = fld.tile([128, 2048], BF16, tag="v_all", name="v_all")

            tog_ctr = [0]

            def synth_group(b, field, g2):
                """4 channels (2 cpairs) -> dest[:, g2*512:(g2+1)*512]."""
                s = st[b]
                BuR, BuI, BvR, BvI = s['B']
                BR, BI = (BuR, BuI) if field == 'u' else (BvR, BvI)
                dest = s['u_all'] if field == 'u' else s['v_all']
                tog = tog_ctr[0]
                tog_ctr[0] += 1
                psG = pp.tile([128, 512], F32, tag=f"bankF{tog % 2}",
                              name="psG")
                for sub in range(2):
                    cpair = 2 * g2 + sub
                    csl = slice(cpair * 128, (cpair + 1) * 128)
                    osl = slice(sub * 256, (sub + 1) * 256)
                    nc.tensor.matmul(psG[:, osl], BR[:, csl], cQF1,
                                     start=True, stop=False)
                    nc.tensor.matmul(psG[:, osl], BI[:, csl], cQF2,
                                     start=False, stop=True)
                G_sb = wk.tile([128, 512], BF16, tag="G_sb", name="G_sb")
                nc.scalar.copy(out=G_sb[:], in_=psG[:])
                psU = pp.tile([128, 512], F32, tag=f"bankF{2 + tog % 2}",
                              name="psU")
                for chl in range(4):
                    sub, chp = chl // 2, chl % 2
                    gr = G_sb[chp * 64:(chp + 1) * 64,
                              sub * 256:sub * 256 + 128]
                    gi = G_sb[chp * 64:(chp + 1) * 64,
                              sub * 256 + 128:sub * 256 + 256]
                    psl = slice(chp * 64, (chp + 1) * 64)
                    osl = slice(chl * 128, (chl + 1) * 128)
                    nc.tensor.matmul(psU[:, osl], cPRT[psl, :], gr,
                                     start=True, stop=False)
                    nc.tensor.matmul(psU[:, osl], cnPIT[psl, :], gi,
                                     start=False, stop=True)
                nc.scalar.copy(out=dest[:, g2 * 512:(g2 + 1) * 512],
                               in_=psU[:])

            def emit_cross_block(b, gI, gJ):
                u_all, v_all, OUT = st[b]['u_all'], st[b]['v_all'], st[b]['OUT']
                W1 = wpp.tile([128, 2048], BF16, tag="W1", name="W1")
                # one 4D-AP product op per W tile: [p, a(4), b(4), 128]
                in0 = _view(u_all[:], gI * 512,
                            [u_all[:].ap[0], [128, 4], [0, 4], [1, 128]])
                in1 = _view(v_all[:], gJ * 512,
                            [v_all[:].ap[0], [0, 4], [128, 4], [1, 128]])
                ps_eng.pick(4.0).tensor_mul(
                    W1[:].rearrange("p (a cb f) -> p a cb f", a=4, cb=4),
                    in0, in1)
                if gI != gJ:
                    W2 = wpp.tile([128, 2048], BF16, tag="W2", name="W2")
                    in0 = _view(u_all[:], gJ * 512,
                                [u_all[:].ap[0], [128, 4], [0, 4], [1, 128]])
                    in1 = _view(v_all[:], gI * 512,
                                [v_all[:].ap[0], [0, 4], [128, 4], [1, 128]])
                    ps_eng.pick(4.0).tensor_mul(
                        W2[:].rearrange("p (bj ca f) -> p bj ca f", bj=4, ca=4),
                        in0, in1)
                    for ai in range(4):
                        a = 4 * gI + ai
                        pch = 8 + _PAIR_IDX[(a, 4 * gJ)]
                        in0 = W1[:, ai * 512:(ai + 1) * 512].rearrange(
                            "p (cb f) -> p cb f", cb=4)
                        in1 = _view(W2[:], ai * 128,
                                    [W2[:].ap[0], [512, 4], [1, 128]])
                        out = _view(OUT[:], pch * 128,
                                    [OUT[:].ap[0], [128, 4], [1, 128]])
                        ps_eng.pick(1.0).tensor_sub(out, in0, in1)
                else:
                    for ai in range(3):
                        a = 4 * gI + ai
                        cnt = 3 - ai
                        pch = 8 + _PAIR_IDX[(a, a + 1)]
                        in0 = _view(W1[:], ai * 512 + (ai + 1) * 128,
                                    [W1[:].ap[0], [128, cnt], [1, 128]])
                        in1 = _view(W1[:], (ai + 1) * 512 + ai * 128,
                                    [W1[:].ap[0], [512, cnt], [1, 128]])
                        out = _view(OUT[:], pch * 128,
                                    [OUT[:].ap[0], [128, cnt], [1, 128]])
                        ps_eng.pick(cnt / 4.0).tensor_sub(out, in0, in1)

            def cross_dma(b, c0, c1):
                OUT = st[b]['OUT']
                nc.gpsimd.dma_start(
                    out=out_sh[b, :, c0:c1, :],
                    in_=OUT[:, c0 * 128:c1 * 128].rearrange(
                        "x (c y) -> x c y", c=c1 - c0))

            # ---------------- interleaved emission across samples ----------
            Bs = list(range(B_PER_CORE))
            for b in Bs:
                stage1(b)
            for b in Bs:
                stage_fr(b)
            for b in Bs:
                stage2(b)
            for b in Bs:
                stage_conv(b)
            for b in Bs:
                stage_B(b)
            # Fine-grained interleave: cross row 0 starts after v0+u0; each
            # further v-group releases the next (0, gJ) block.  Rows gI>=1
            # follow with their own u-group.  DMA chunks per row:
            # gI=0 covers pch 8..61, gI=1 -> ..99, gI=2 -> ..121, gI=3 -> ..127.
            chunk_hi = [62, 100, 122, 128]
            for b in Bs:
                synth_group(b, 'v', 0)
            for b in Bs:
                synth_group(b, 'u', 0)
            for b in Bs:
                emit_cross_block(b, 0, 0)
            for gJ in range(1, 4):
                for b in Bs:
                    synth_group(b, 'v', gJ)
                for b in Bs:
                    emit_cross_block(b, 0, gJ)
            if 'dma' not in ablate:
                for b in Bs:
                    cross_dma(b, 8, chunk_hi[0])
            chunk_lo = chunk_hi[0]
            for gI in range(1, 4):
                for b in Bs:
                    synth_group(b, 'u', gI)
                for gJ in range(gI, 4):
                    for b in Bs:
                        emit_cross_block(b, gI, gJ)
                if 'dma' not in ablate:
                    for b in Bs:
                        cross_dma(b, chunk_lo, chunk_hi[gI])
                chunk_lo = chunk_hi[gI]
    nc.compile()
    return nc


# ---------------------------------------------------------------------------
# entry point
# ---------------------------------------------------------------------------

_PROGRAM = {}


def _get_program(reps=1, ablate=(), **kw):
    key = (reps, tuple(sorted(ablate)), tuple(sorted(kw.items())))
    if key not in _PROGRAM:
        _PROGRAM[key] = build_program(reps, ablate=ablate, **kw)
    return _PROGRAM[key]


LAST_EXEC_NS = None
LAST_RESULT = None


def kernel(f, kernel):
    global LAST_EXEC_NS, LAST_RESULT
    f_bf = np.ascontiguousarray(np.asarray(f), dtype=np.float32).astype(NPBF16)
    k_all = _prep_k_all(np.asarray(kernel))
    blob, _ = _host_consts()
    nc = _get_program()
    in_maps = [
        {"f_in": f_bf[2 * c:2 * c + 2], "k_all": k_all, "cb": blob}
        for c in range(N_CORES)
    ]
    import os
    trace = bool(os.environ.get("KERNEL_TRACE"))
    res = run_bass_kernel_spmd(nc, in_maps, list(range(N_CORES)), trace=trace)
    LAST_RESULT = res
    if res.exec_time_ns is not None:
        LAST_EXEC_NS = res.exec_time_ns
    out = np.concatenate([res.results[c]["out_sh"] for c in range(N_CORES)], axis=0)
    # device layout is [b, X, ch, Y]; return the [b, ch, X, Y] view
    return out.transpose(0, 2, 1, 3)


# revision 20
# speedup vs baseline: 1.1134x; 1.0380x over previous
"""Trainium2 Bass kernel for nn_EquivariantLayer (spectral equivariant layer).

Data-parallel over batch: 2 samples/core x 8 cores. All-bf16 pipeline:

  stage1:  psA = f^T @ [ExR^T|ExI^T|-ExR^T | Rx^T]   (one fused matmul/2ch)
  stage2:  F = Ey @ A       (c-major conv layout via tile_position packing)
  conv:    M = F (*) K elementwise (K real); i-reduction via selector matmul
  uncurl:  pure-imaginary TO_U/TO_V -> real mults by t/s tables
  synth:   per channel pair: G = B @ QF (Q-side), field = P @ G (P-side)
  fr:      direct 2x Fourier upsample fr_i = Rx @ f_i @ Cy^T
  cross:   u_a v_b - u_b v_a in bf16 on DVE/Pool, written straight into a
           per-sample output tile; SWDGE (gpsimd) DMAs cast bf16->f32 on the
           way out to HBM.

All matmul operands are bf16 (1 cycle/row on PE vs 4 for fp32); every
PSUM->SBUF copy casts f32 accumulators down to bf16. Output 16.8MB f32 per
core dominates DMA time; compute is sized to hide beneath it.
"""
import sys
import numpy as np
import ml_dtypes

if '/opt/trn_rl_repo' not in sys.path:
    sys.path.insert(0, '/opt/trn_rl_repo')

import concourse.bass as bass
from concourse import bacc
import concourse.mybir as mybir
import concourse.tile as tile
from concourse.bass import AP
from concourse.bass_utils import run_bass_kernel_spmd

F32 = mybir.dt.float32
BF16 = mybir.dt.bfloat16
NPBF16 = ml_dtypes.bfloat16
N_CORES = 8
B_PER_CORE = 2
C1, C2, N1, N2 = 8, 16, 64, 128
NCH_OUT = 128  # 8 fr + 120 cross

I_IDX, J_IDX = np.triu_indices(C2, 1)
_PAIR_IDX = {}
for _p, (_a, _b) in enumerate(zip(I_IDX, J_IDX)):
    _PAIR_IDX[(int(_a), int(_b))] = _p


# ---------------------------------------------------------------------------
# host-side constant construction
# ---------------------------------------------------------------------------

def _host_consts():
    x = np.arange(64)
    kx = np.arange(64)
    c = np.arange(32)
    y = np.arange(64)
    X = np.arange(128)
    Y = np.arange(128)

    FRs = np.where(kx <= 32, kx, kx - 64).astype(np.float64)  # signed row freq

    ExR = np.cos(2 * np.pi * np.outer(kx, x) / 64)   # [kx, x]
    ExI = -np.sin(2 * np.pi * np.outer(kx, x) / 64)
    ExF = np.concatenate([ExR.T, ExI.T, -ExR.T], axis=1)   # [x, 192]

    EyCT = np.cos(2 * np.pi * np.outer(c, y) / 64).T   # [y=64, c=32]
    EyST = np.sin(2 * np.pi * np.outer(c, y) / 64).T
    EyCT2 = np.concatenate([EyCT, EyCT], axis=0)       # [128, 32] doubled rows
    EyST2 = np.concatenate([EyST, EyST], axis=0)

    S_sel = np.zeros((128, 32))
    for im in range(4):
        S_sel[im * 32 + np.arange(32), np.arange(32)] = 1.0

    den = FRs[None, :] ** 2 + c[:, None].astype(np.float64) ** 2
    den[0, 0] = 1.0
    t_u = c[:, None] / den                           # [32, 64]
    s_v = -FRs[None, :] / den
    t_rep = np.tile(t_u, (1, 8))                     # [32, 512] (j-rep)
    s_rep = np.tile(s_v, (1, 8))
    tsg = np.concatenate([-t_rep, t_rep, -s_rep, s_rep], axis=1)  # [32, 2048]

    w_c = np.where(c == 0, 1.0, 2.0)
    s_q = 2.0 / (128.0 * 128.0)
    QRT = (s_q * w_c[None, :] * np.cos(2 * np.pi * np.outer(Y, c) / 128)).T  # [c, Y]
    QIT = (s_q * w_c[None, :] * np.sin(2 * np.pi * np.outer(Y, c) / 128)).T
    QF1 = np.concatenate([QRT, QIT], axis=1)         # [32, 256]
    QF2 = np.concatenate([-QIT, QRT], axis=1)

    PRT = np.cos(2 * np.pi * np.outer(FRs, X) / 128)   # [kx=64, X=128]
    PIT = np.sin(2 * np.pi * np.outer(FRs, X) / 128)
    PRT[32, :] = 0.0
    PIT[32, :] = 0.0
    # doubled rows so lhsT slices can match rhs base partition 0 or 64
    PRT2 = np.concatenate([PRT, PRT], axis=0)          # [128, 128]
    nPIT2 = np.concatenate([-PIT, -PIT], axis=0)

    # direct fr path: fr_i = Rx @ f_i @ Cy^T (pure 2x Fourier upsampling)
    ExRm = np.cos(2 * np.pi * np.outer(kx, x) / 64)
    ExIm = -np.sin(2 * np.pi * np.outer(kx, x) / 64)
    EyRm = np.cos(2 * np.pi * np.outer(c, y) / 64)
    EyIm = -np.sin(2 * np.pi * np.outer(c, y) / 64)
    QRm = s_q * w_c[None, :] * np.cos(2 * np.pi * np.outer(Y, c) / 128)
    QIm = s_q * w_c[None, :] * np.sin(2 * np.pi * np.outer(Y, c) / 128)
    Rx = PRT.T @ ExRm - PIT.T @ ExIm                 # [128, 64]
    Cy = QRm @ EyRm - QIm @ EyIm                     # [128, 64]
    RxT = Rx.T                                       # [x=64, X=128]
    CyT = np.concatenate([Cy.T, Cy.T], axis=0)       # [128, 128] doubled rows

    ExFT1 = np.concatenate([ExF, RxT], axis=1)       # [64, 320]

    # pack all consts into one [128, W] bf16 blob: (rows, width, offset)
    consts = dict(ExFT1=ExFT1, EyCT=EyCT2, EyST=EyST2, S_sel=S_sel, tsg=tsg,
                  QF1=QF1, QF2=QF2, PRT=PRT2, nPIT=nPIT2, CyT=CyT)
    layout = {}
    off = 0
    for name, arr in consts.items():
        layout[name] = (arr.shape[0], arr.shape[1], off)
        off += arr.shape[1]
    blob = np.zeros((128, off), dtype=NPBF16)
    for name, arr in consts.items():
        r, w, o = layout[name]
        blob[:r, o:o + w] = arr.astype(NPBF16)
    return blob, layout


def _rot90_kernel(k):
    y = np.swapaxes(k, -2, -1)
    return np.concatenate([y[..., :1], y[..., :0:-1]], axis=-1)


def _symmetric_kernel(k):
    k1 = k
    k2 = _rot90_kernel(k1)
    k3 = _rot90_kernel(k2)
    k4 = _rot90_kernel(k3)
    k5 = np.swapaxes(k1, -2, -1)
    k6 = _rot90_kernel(k5)
    k7 = _rot90_kernel(k6)
    k8 = _rot90_kernel(k7)
    return (k1 + k2 + k3 + k4 + k5 + k6 + k7 + k8) / 8.0


def _prep_k_all(kernel_np):
    """kernel [1,8,16,64,64] -> k_all [128, 2048] conv-layout packed (bf16)."""
    ksym = _symmetric_kernel(kernel_np.astype(np.float64))[0]   # [8,16,64,64]
    K = np.fft.rfft2(ksym).real                                  # [8,16,64,33]
    Kc = np.transpose(K[:, :, :, :32], (0, 1, 3, 2)).copy()      # [i,j,c,kx]
    Kc[:, :, :, 32] = 0.0                                        # kx nyquist
    k_all = np.zeros((128, 2048), dtype=NPBF16)
    for i in range(8):
        h, im = i // 4, i % 4
        for j in range(16):
            k_all[im * 32:(im + 1) * 32, j * 128 + h * 64: j * 128 + h * 64 + 64] = \
                Kc[i, j].astype(NPBF16)
    return k_all


# ---------------------------------------------------------------------------
# device program
# ---------------------------------------------------------------------------

def _bcast(ap, n, axis_pos=1):
    """Insert a zero-step broadcast dim of size n into an AP."""
    dims = list(ap.ap)
    dims.insert(axis_pos, [0, n])
    return AP(ap.tensor, ap.offset, dims)


def _view(ap, offset_elems, dims):
    """Raw AP view on the same tensor: explicit offset (elems) + [step, count]."""
    return AP(ap.tensor, ap.offset + offset_elems, dims)


class EngSched:
    """Greedy engine load balancer: pick the engine minimizing accumulated
    busy-ns + this op's cost on that engine."""

    def __init__(self, engmap):
        self.eng = engmap
        self.acc = {k: 0.0 for k in engmap}

    def pick(self, costs):
        k = min(costs, key=lambda k: self.acc[k] + costs[k])
        self.acc[k] += costs[k]
        return self.eng[k]

    def charge(self, k, cost):
        self.acc[k] += cost


def build_program(reps=1, ablate=()):
    nc = bacc.Bacc("TRN2", target_bir_lowering=False)
    blob, lay = _host_consts()

    f_in = nc.dram_tensor("f_in", [B_PER_CORE, C1, 64, 64], BF16, kind="ExternalInput")
    k_in = nc.dram_tensor("k_all", [128, 2048], BF16, kind="ExternalInput")
    cb_in = nc.dram_tensor("cb", list(blob.shape), BF16, kind="ExternalInput")
    # transposed output layout [b, X, ch, Y]; host returns .transpose(0,2,1,3)
    out_sh = nc.dram_tensor("out_sh", [B_PER_CORE, 128, NCH_OUT, 128], F32,
                            kind="ExternalOutput")

    import os
    wv = float(os.environ.get("KWV", "1.6"))   # DVE weight for prod/sub split
    wp = float(os.environ.get("KWP", "1.0"))   # Pool weight

    with tile.TileContext(nc) as tc:
        with (
            tc.tile_pool(name="cp", bufs=1) as cp,
            tc.tile_pool(name="fld", bufs=2) as fld,     # per-sample u/v/out
            tc.tile_pool(name="wk", bufs=2) as wk,       # small working tiles
            tc.tile_pool(name="mw", bufs=2) as mwp,      # conv wide tiles
            tc.tile_pool(name="wp", bufs=2) as wpp,      # cross product blocks
            tc.tile_pool(name="pp", bufs=1, space="PSUM") as pp,
        ):
            # ---- load constants (one blob DMA + k) ----
            CB = cp.tile(list(blob.shape), BF16, tag="CB", name="CB")
            nc.sync.dma_start(out=CB[:], in_=cb_in[:])
            k_sb = cp.tile([128, 2048], BF16, tag="k_sb", name="k_sb")
            nc.sync.dma_start(out=k_sb[:], in_=k_in[:])

            def cview(name):
                r, w, o = lay[name]
                return CB[0:r, o:o + w]

            cExFT1 = cview("ExFT1")
            cEyCT = cview("EyCT")
            cEyST = cview("EyST")
            cS_sel = cview("S_sel")
            ctsg = cview("tsg")
            cQF1 = cview("QF1")
            cQF2 = cview("QF2")
            cPRT = cview("PRT")
            cnPIT = cview("nPIT")
            cCyT = cview("CyT")

            # global greedy balancer across DVE / ACT / Pool
    
            es = EngSched({'v': nc.vector, 'a': nc.scalar, 'p': nc.gpsimd})

            def cp_copy(out, in_, free):
                # PSUM->SBUF copy; cost model: ACT 0.833/el + bubble,
                # DVE 1.042/el + bubble, Pool 0.833/el (no errata bubble)
                eng = es.pick({'a': free * 0.833 + 190,
                               'v': free * 1.042 + 180,
                               'p': free * 0.833 + 80})
                if eng is nc.scalar:
                    eng.copy(out=out, in_=in_)
                else:
                    eng.tensor_copy(out, in_)

            st = {b: {} for b in range(B_PER_CORE)}

            def stage1(b):
                s = st[b]
                s['OUT'] = fld.tile([128, NCH_OUT * 128], BF16, tag="OUT",
                                    name="OUT")
                fsb = wk.tile([64, 512], BF16, tag="fsb", name="fsb")
                nc.sync.dma_start(
                    out=fsb[:].rearrange("x (i y) -> x i y", i=8),
                    in_=f_in[b].rearrange("i x y -> x i y"))
                A_sb = []
                for ip in range(4):
                    psA = pp.tile([128, 320], F32, tag="bankA", bufs=2, name="psA")
                    nc.tensor.matmul(psA[:], fsb[:, ip * 128:(ip + 1) * 128],
                                     cExFT1, start=True, stop=True)
                    a_t = wk.tile([128, 320], BF16, tag=f"a{ip}", name="a_t")
                    cp_copy(a_t[:], psA[:], 320)
                    A_sb.append(a_t)
                s['A_sb'] = A_sb

            def stage_fr(b):
                s = st[b]
                A_sb, OUT = s['A_sb'], s['OUT']
                for iph in range(4):   # 2 channels per psUf tile
                    psUf = pp.tile([128, 256], F32, tag="bankA", bufs=2, name="psUf")
                    for iloc2 in range(2):
                        i = 2 * iph + iloc2
                        ip, iloc = i // 2, i % 2
                        t1 = A_sb[ip][iloc * 64:(iloc + 1) * 64, 192:320]
                        nc.tensor.matmul(psUf[:, iloc2 * 128:(iloc2 + 1) * 128],
                                         t1, cCyT[iloc * 64:(iloc + 1) * 64, :],
                                         start=True, stop=True)
                    cp_copy(OUT[:, iph * 256:(iph + 1) * 256], psUf[:], 256)
                if 'dma' not in ablate:
                    nc.gpsimd.dma_start(
                        out=out_sh[b, :, 0:8, :],
                        in_=OUT[:, 0:1024].rearrange("x (c y) -> x c y", c=8))

            def stage2(b):
                s = st[b]
                A_sb = s['A_sb']
                psFcv = [pp.tile([128, 128], F32, tag=f"bankF{4 + h}", name="psFcv")
                         for h in range(2)]
                for i in range(8):
                    iloc = i % 2
                    ysl = slice(iloc * 64, (iloc + 1) * 64)
                    A_RI = A_sb[i // 2][ysl, 0:128]
                    A_IS = A_sb[i // 2][ysl, 64:192]
                    h, im = i // 4, i % 4
                    sl = slice(im * 32, (im + 1) * 32)
                    tp = (0, im * 32)
                    nc.tensor.matmul(psFcv[h][sl, :], cEyCT[ysl, :], A_RI,
                                     start=True, stop=False, tile_position=tp)
                    nc.tensor.matmul(psFcv[h][sl, :], cEyST[ysl, :], A_IS,
                                     start=False, stop=True, tile_position=tp)
                Fcv = wk.tile([128, 256], BF16, tag="Fcv", name="Fcv")
                for h in range(2):
                    cp_copy(_view(Fcv[:], h * 64,
                                  [Fcv[:].ap[0], [128, 2], [1, 64]]),
                            psFcv[h][:].rearrange("p (r k) -> p r k", r=2), 128)
                s['Fcv'] = Fcv

            def stage_conv(b):
                s = st[b]
                Fcv = s['Fcv']
                Mw = []
                for RI in range(2):
                    m_t = mwp.tile([128, 2048], BF16, tag=f"mw{RI}", name="m_t")
                    conv_eng = es.pick({'v': 2048 * 0.52 + 60,
                                        'p': 2048 * 0.833 + 60})
                    conv_eng.tensor_mul(
                        m_t[:].rearrange("p (j f) -> p j f", j=16),
                        _bcast(Fcv[:, RI * 128:(RI + 1) * 128], 16),
                        k_sb[:].rearrange("p (j f) -> p j f", j=16))
                    Mw.append(m_t)

                acv_sb = wk.tile([32, 2048], BF16, tag="acv", name="acv_sb")
                for RI in range(2):
                    for jh in range(2):
                        ps_acv = pp.tile([32, 512], F32, tag="bankA", bufs=2,
                                         name="ps_acv")
                        for h in range(2):
                            rhs = _view(Mw[RI][:], jh * 1024 + h * 64,
                                        [Mw[RI][:].ap[0], [128, 8], [1, 64]])
                            nc.tensor.matmul(ps_acv[:], cS_sel, rhs,
                                             start=(h == 0), stop=(h == 1))
                        cp_copy(
                            acv_sb[:, (RI * 2 + jh) * 512:(RI * 2 + jh + 1) * 512],
                            ps_acv[:], 512)
                s['acv_sb'] = acv_sb

            def stage_B(b):
                s = st[b]
                acv_sb = s['acv_sb']
                # ---------------- uncurl: B = acv (*) t/s ----------------
                BuR = wk.tile([32, 1024], BF16, tag="BuR", name="BuR")
                BuI = wk.tile([32, 1024], BF16, tag="BuI", name="BuI")
                BvR = wk.tile([32, 1024], BF16, tag="BvR", name="BvR")
                BvI = wk.tile([32, 1024], BF16, tag="BvI", name="BvI")
                # acv layout: [R jh0 | R jh1 | I jh0 | I jh1] each 512
                # tsg: [-t | t | -s | s] each 512 (j-repeated, j-independent)
                for RI in range(2):
                    a_v = _view(acv_sb[:], RI * 1024,
                                [acv_sb[:].ap[0], [512, 2], [1, 512]])
                    bc = {'v': 1024 * 0.52 + 60, 'p': 1024 * 0.833 + 60}
                    if RI == 0:  # A_R -> imag parts (mult by +t / +s)
                        es.pick(bc).tensor_mul(
                            BuI[:].rearrange("p (j f) -> p j f", j=2),
                            a_v, _bcast(ctsg[:, 512:1024], 2))
                        es.pick(bc).tensor_mul(
                            BvI[:].rearrange("p (j f) -> p j f", j=2),
                            a_v, _bcast(ctsg[:, 1536:2048], 2))
                    else:        # A_I -> real parts (mult by -t / -s)
                        es.pick(bc).tensor_mul(
                            BuR[:].rearrange("p (j f) -> p j f", j=2),
                            a_v, _bcast(ctsg[:, 0:512], 2))
                        es.pick(bc).tensor_mul(
                            BvR[:].rearrange("p (j f) -> p j f", j=2),
                            a_v, _bcast(ctsg[:, 1024:1536], 2))
                s['B'] = (BuR, BuI, BvR, BvI)
                s['u_all'] = fld.tile([128, 2048], BF16, tag="u_all", name="u_all")
                s['v_all'] = fld.tile([128, 2048], BF16, tag="v_all", name="v_all")

            tog_ctr = [0]

            def synth_group(b, field, g2):
                """4 channels (2 cpairs) -> dest[:, g2*512:(g2+1)*512]."""
                s = st[b]
                BuR, BuI, BvR, BvI = s['B']
                BR, BI = (BuR, BuI) if field == 'u' else (BvR, BvI)
                dest = s['u_all'] if field == 'u' else s['v_all']
                tog = tog_ctr[0]
                tog_ctr[0] += 1
                psG = pp.tile([128, 512], F32, tag=f"bankF{tog % 2}",
                              name="psG")
                for sub in range(2):
                    cpair = 2 * g2 + sub
                    csl = slice(cpair * 128, (cpair + 1) * 128)
                    osl = slice(sub * 256, (sub + 1) * 256)
                    nc.tensor.matmul(psG[:, osl], BR[:, csl], cQF1,
                                     start=True, stop=False)
                    nc.tensor.matmul(psG[:, osl], BI[:, csl], cQF2,
                                     start=False, stop=True)
                G_sb = wk.tile([128, 512], BF16, tag="G_sb", name="G_sb")
                cp_copy(G_sb[:], psG[:], 512)
                psU = pp.tile([128, 512], F32, tag=f"bankF{2 + tog % 2}",
                              name="psU")
                for chl in range(4):
                    sub, chp = chl // 2, chl % 2
                    gr = G_sb[chp * 64:(chp + 1) * 64,
                              sub * 256:sub * 256 + 128]
                    gi = G_sb[chp * 64:(chp + 1) * 64,
                              sub * 256 + 128:sub * 256 + 256]
                    psl = slice(chp * 64, (chp + 1) * 64)
                    osl = slice(chl * 128, (chl + 1) * 128)
                    nc.tensor.matmul(psU[:, osl], cPRT[psl, :], gr,
                                     start=True, stop=False)
                    nc.tensor.matmul(psU[:, osl], cnPIT[psl, :], gi,
                                     start=False, stop=True)
                cp_copy(dest[:, g2 * 512:(g2 + 1) * 512], psU[:], 512)

            def emit_cross_block(b, gI, gJ):
                u_all, v_all, OUT = st[b]['u_all'], st[b]['v_all'], st[b]['OUT']
                W1 = wpp.tile([128, 2048], BF16, tag="W1", name="W1")
                # one 4D-AP product op per W tile: [p, a(4), b(4), 128]
                in0 = _view(u_all[:], gI * 512,
                            [u_all[:].ap[0], [128, 4], [0, 4], [1, 128]])
                in1 = _view(v_all[:], gJ * 512,
                            [v_all[:].ap[0], [0, 4], [128, 4], [1, 128]])
                es.pick({'v': 2048 * 0.52 + 60,
                         'p': 2048 * 0.833 + 60}).tensor_mul(
                    W1[:].rearrange("p (a cb f) -> p a cb f", a=4, cb=4),
                    in0, in1)
                if gI != gJ:
                    W2 = wpp.tile([128, 2048], BF16, tag="W2", name="W2")
                    in0 = _view(u_all[:], gJ * 512,
                                [u_all[:].ap[0], [128, 4], [0, 4], [1, 128]])
                    in1 = _view(v_all[:], gI * 512,
                                [v_all[:].ap[0], [0, 4], [128, 4], [1, 128]])
                    es.pick({'v': 2048 * 0.52 + 60,
                             'p': 2048 * 0.833 + 60}).tensor_mul(
                        W2[:].rearrange("p (bj ca f) -> p bj ca f", bj=4, ca=4),
                        in0, in1)
                    for ai in range(4):
                        a = 4 * gI + ai
                        pch = 8 + _PAIR_IDX[(a, 4 * gJ)]
                        in0 = W1[:, ai * 512:(ai + 1) * 512].rearrange(
                            "p (cb f) -> p cb f", cb=4)
                        in1 = _view(W2[:], ai * 128,
                                    [W2[:].ap[0], [512, 4], [1, 128]])
                        out = _view(OUT[:], pch * 128,
                                    [OUT[:].ap[0], [128, 4], [1, 128]])
                        es.pick({'v': 512 * 0.52 + 60,
                                 'p': 512 * 0.833 + 60}).tensor_sub(out, in0, in1)
                else:
                    for ai in range(3):
                        a = 4 * gI + ai
                        cnt = 3 - ai
                        pch = 8 + _PAIR_IDX[(a, a + 1)]
                        in0 = _view(W1[:], ai * 512 + (ai + 1) * 128,
                                    [W1[:].ap[0], [128, cnt], [1, 128]])
                        in1 = _view(W1[:], (ai + 1) * 512 + ai * 128,
                                    [W1[:].ap[0], [512, cnt], [1, 128]])
                        out = _view(OUT[:], pch * 128,
                                    [OUT[:].ap[0], [128, cnt], [1, 128]])
                        es.pick({'v': cnt * 128 * 0.52 + 60,
                                 'p': cnt * 128 * 0.833 + 60}).tensor_sub(
                            out, in0, in1)

            def cross_dma(b, c0, c1):
                OUT = st[b]['OUT']
                es.charge('p', 1100.0)
                nc.gpsimd.dma_start(
                    out=out_sh[b, :, c0:c1, :],
                    in_=OUT[:, c0 * 128:c1 * 128].rearrange(
                        "x (c y) -> x c y", c=c1 - c0))

            # ---------------- interleaved emission across samples ----------
            Bs = list(range(B_PER_CORE))
            for b in Bs:
                stage1(b)
            for b in Bs:
                stage_fr(b)
            for b in Bs:
                stage2(b)
            for b in Bs:
                stage_conv(b)
            for b in Bs:
                stage_B(b)
            # Fine-grained interleave: cross row 0 starts after v0+u0; each
            # further v-group releases the next (0, gJ) block.  Rows gI>=1
            # follow with their own u-group.  DMA chunks per row:
            # gI=0 covers pch 8..61, gI=1 -> ..99, gI=2 -> ..121, gI=3 -> ..127.
            chunk_hi = [62, 100, 122, 128]
            for b in Bs:
                synth_group(b, 'v', 0)
            for b in Bs:
                synth_group(b, 'u', 0)
            for b in Bs:
                emit_cross_block(b, 0, 0)
            for gJ in range(1, 4):
                for b in Bs:
                    synth_group(b, 'v', gJ)
                for b in Bs:
                    emit_cross_block(b, 0, gJ)
            if 'dma' not in ablate:
                for b in Bs:
                    cross_dma(b, 8, chunk_hi[0])
            chunk_lo = chunk_hi[0]
            for gI in range(1, 4):
                for b in Bs:
                    synth_group(b, 'u', gI)
                for gJ in range(gI, 4):
                    for b in Bs:
                        emit_cross_block(b, gI, gJ)
                if 'dma' not in ablate:
                    for b in Bs:
                        cross_dma(b, chunk_lo, chunk_hi[gI])
                chunk_lo = chunk_hi[gI]
    nc.compile()
    return nc


# ---------------------------------------------------------------------------
# entry point
# ---------------------------------------------------------------------------

_PROGRAM = {}


def _get_program(reps=1, ablate=(), **kw):
    key = (reps, tuple(sorted(ablate)), tuple(sorted(kw.items())))
    if key not in _PROGRAM:
        _PROGRAM[key] = build_program(reps, ablate=ablate, **kw)
    return _PROGRAM[key]


LAST_EXEC_NS = None
LAST_RESULT = None


def kernel(f, kernel):
    global LAST_EXEC_NS, LAST_RESULT
    f_bf = np.ascontiguousarray(np.asarray(f), dtype=np.float32).astype(NPBF16)
    k_all = _prep_k_all(np.asarray(kernel))
    blob, _ = _host_consts()
    nc = _get_program()
    in_maps = [
        {"f_in": f_bf[2 * c:2 * c + 2], "k_all": k_all, "cb": blob}
        for c in range(N_CORES)
    ]
    import os
    trace = bool(os.environ.get("KERNEL_TRACE"))
    res = run_bass_kernel_spmd(nc, in_maps, list(range(N_CORES)), trace=trace)
    LAST_RESULT = res
    if res.exec_time_ns is not None:
        LAST_EXEC_NS = res.exec_time_ns
    out = np.concatenate([res.results[c]["out_sh"] for c in range(N_CORES)], axis=0)
    # device layout is [b, X, ch, Y]; return the [b, ch, X, Y] view
    return out.transpose(0, 2, 1, 3)


# revision 21
# speedup vs baseline: 1.1491x; 1.0321x over previous
"""Trainium2 Bass kernel for nn_EquivariantLayer (spectral equivariant layer).

Data-parallel over batch: 2 samples/core x 8 cores. All-bf16 pipeline:

  stage1:  psA = f^T @ [ExR^T|ExI^T|-ExR^T | Rx^T]   (one fused matmul/2ch)
  stage2:  F = Ey @ A       (c-major conv layout via tile_position packing)
  conv:    M = F (*) K elementwise (K real); i-reduction via selector matmul
  uncurl:  pure-imaginary TO_U/TO_V -> real mults by t/s tables
  synth:   per channel pair: G = B @ QF (Q-side), field = P @ G (P-side)
  fr:      direct 2x Fourier upsample fr_i = Rx @ f_i @ Cy^T
  cross:   u_a v_b - u_b v_a in bf16 on DVE/Pool, written straight into a
           per-sample output tile; SWDGE (gpsimd) DMAs cast bf16->f32 on the
           way out to HBM.

All matmul operands are bf16 (1 cycle/row on PE vs 4 for fp32); every
PSUM->SBUF copy casts f32 accumulators down to bf16. Output 16.8MB f32 per
core dominates DMA time; compute is sized to hide beneath it.
"""
import sys
import numpy as np
import ml_dtypes

if '/opt/trn_rl_repo' not in sys.path:
    sys.path.insert(0, '/opt/trn_rl_repo')

import concourse.bass as bass
from concourse import bacc
import concourse.mybir as mybir
import concourse.tile as tile
from concourse.bass import AP
from concourse.bass_utils import run_bass_kernel_spmd

F32 = mybir.dt.float32
BF16 = mybir.dt.bfloat16
NPBF16 = ml_dtypes.bfloat16
N_CORES = 8
B_PER_CORE = 2
C1, C2, N1, N2 = 8, 16, 64, 128
NCH_OUT = 128  # 8 fr + 120 cross

I_IDX, J_IDX = np.triu_indices(C2, 1)
_PAIR_IDX = {}
for _p, (_a, _b) in enumerate(zip(I_IDX, J_IDX)):
    _PAIR_IDX[(int(_a), int(_b))] = _p


# ---------------------------------------------------------------------------
# host-side constant construction
# ---------------------------------------------------------------------------

def _host_consts():
    x = np.arange(64)
    kx = np.arange(64)
    c = np.arange(32)
    y = np.arange(64)
    X = np.arange(128)
    Y = np.arange(128)

    FRs = np.where(kx <= 32, kx, kx - 64).astype(np.float64)  # signed row freq

    ExR = np.cos(2 * np.pi * np.outer(kx, x) / 64)   # [kx, x]
    ExI = -np.sin(2 * np.pi * np.outer(kx, x) / 64)
    ExF = np.concatenate([ExR.T, ExI.T, -ExR.T], axis=1)   # [x, 192]

    EyCT = np.cos(2 * np.pi * np.outer(c, y) / 64).T   # [y=64, c=32]
    EyST = np.sin(2 * np.pi * np.outer(c, y) / 64).T
    EyCT2 = np.concatenate([EyCT, EyCT], axis=0)       # [128, 32] doubled rows
    EyST2 = np.concatenate([EyST, EyST], axis=0)

    S_sel = np.zeros((128, 32))
    for im in range(4):
        S_sel[im * 32 + np.arange(32), np.arange(32)] = 1.0

    den = FRs[None, :] ** 2 + c[:, None].astype(np.float64) ** 2
    den[0, 0] = 1.0
    t_u = c[:, None] / den                           # [32, 64]
    s_v = -FRs[None, :] / den
    t_rep = np.tile(t_u, (1, 8))                     # [32, 512] (j-rep)
    s_rep = np.tile(s_v, (1, 8))
    tsg = np.concatenate([-t_rep, t_rep, -s_rep, s_rep], axis=1)  # [32, 2048]

    w_c = np.where(c == 0, 1.0, 2.0)
    s_q = 2.0 / (128.0 * 128.0)
    QRT = (s_q * w_c[None, :] * np.cos(2 * np.pi * np.outer(Y, c) / 128)).T  # [c, Y]
    QIT = (s_q * w_c[None, :] * np.sin(2 * np.pi * np.outer(Y, c) / 128)).T
    QF1 = np.concatenate([QRT, QIT], axis=1)         # [32, 256]
    QF2 = np.concatenate([-QIT, QRT], axis=1)

    PRT = np.cos(2 * np.pi * np.outer(FRs, X) / 128)   # [kx=64, X=128]
    PIT = np.sin(2 * np.pi * np.outer(FRs, X) / 128)
    PRT[32, :] = 0.0
    PIT[32, :] = 0.0
    # doubled rows so lhsT slices can match rhs base partition 0 or 64
    PRT2 = np.concatenate([PRT, PRT], axis=0)          # [128, 128]
    nPIT2 = np.concatenate([-PIT, -PIT], axis=0)

    # direct fr path: fr_i = Rx @ f_i @ Cy^T (pure 2x Fourier upsampling)
    ExRm = np.cos(2 * np.pi * np.outer(kx, x) / 64)
    ExIm = -np.sin(2 * np.pi * np.outer(kx, x) / 64)
    EyRm = np.cos(2 * np.pi * np.outer(c, y) / 64)
    EyIm = -np.sin(2 * np.pi * np.outer(c, y) / 64)
    QRm = s_q * w_c[None, :] * np.cos(2 * np.pi * np.outer(Y, c) / 128)
    QIm = s_q * w_c[None, :] * np.sin(2 * np.pi * np.outer(Y, c) / 128)
    Rx = PRT.T @ ExRm - PIT.T @ ExIm                 # [128, 64]
    Cy = QRm @ EyRm - QIm @ EyIm                     # [128, 64]
    RxT = Rx.T                                       # [x=64, X=128]
    CyT = np.concatenate([Cy.T, Cy.T], axis=0)       # [128, 128] doubled rows

    ExFT1 = np.concatenate([ExF, RxT], axis=1)       # [64, 320]

    # pack consts into two [128, W] bf16 blobs: CB1 = front-stage consts
    # (small, loads fast so stage1 starts early), CB2 = the rest.
    blobs = []
    layout = {}
    for bi, consts in enumerate([
        dict(ExFT1=ExFT1, EyCT=EyCT2, EyST=EyST2, S_sel=S_sel, CyT=CyT),
        dict(tsg=tsg, QF1=QF1, QF2=QF2, PRT=PRT2, nPIT=nPIT2),
    ]):
        off = 0
        for name, arr in consts.items():
            layout[name] = (bi, arr.shape[0], arr.shape[1], off)
            off += arr.shape[1]
        blob = np.zeros((128, off), dtype=NPBF16)
        for name, arr in consts.items():
            _, r, w, o = layout[name]
            blob[:r, o:o + w] = arr.astype(NPBF16)
        blobs.append(blob)
    return blobs, layout


def _rot90_kernel(k):
    y = np.swapaxes(k, -2, -1)
    return np.concatenate([y[..., :1], y[..., :0:-1]], axis=-1)


def _symmetric_kernel(k):
    k1 = k
    k2 = _rot90_kernel(k1)
    k3 = _rot90_kernel(k2)
    k4 = _rot90_kernel(k3)
    k5 = np.swapaxes(k1, -2, -1)
    k6 = _rot90_kernel(k5)
    k7 = _rot90_kernel(k6)
    k8 = _rot90_kernel(k7)
    return (k1 + k2 + k3 + k4 + k5 + k6 + k7 + k8) / 8.0


def _prep_k_all(kernel_np):
    """kernel [1,8,16,64,64] -> k_all [128, 2048] conv-layout packed (bf16)."""
    ksym = _symmetric_kernel(kernel_np.astype(np.float64))[0]   # [8,16,64,64]
    K = np.fft.rfft2(ksym).real                                  # [8,16,64,33]
    Kc = np.transpose(K[:, :, :, :32], (0, 1, 3, 2)).copy()      # [i,j,c,kx]
    Kc[:, :, :, 32] = 0.0                                        # kx nyquist
    k_all = np.zeros((128, 2048), dtype=NPBF16)
    for i in range(8):
        h, im = i // 4, i % 4
        for j in range(16):
            k_all[im * 32:(im + 1) * 32, j * 128 + h * 64: j * 128 + h * 64 + 64] = \
                Kc[i, j].astype(NPBF16)
    return k_all


# ---------------------------------------------------------------------------
# device program
# ---------------------------------------------------------------------------

def _bcast(ap, n, axis_pos=1):
    """Insert a zero-step broadcast dim of size n into an AP."""
    dims = list(ap.ap)
    dims.insert(axis_pos, [0, n])
    return AP(ap.tensor, ap.offset, dims)


def _view(ap, offset_elems, dims):
    """Raw AP view on the same tensor: explicit offset (elems) + [step, count]."""
    return AP(ap.tensor, ap.offset + offset_elems, dims)


class EngSched:
    """Greedy engine load balancer: pick the engine minimizing accumulated
    busy-ns + this op's cost on that engine."""

    def __init__(self, engmap):
        self.eng = engmap
        self.acc = {k: 0.0 for k in engmap}

    def pick(self, costs):
        k = min(costs, key=lambda k: self.acc[k] + costs[k])
        self.acc[k] += costs[k]
        return self.eng[k]

    def charge(self, k, cost):
        self.acc[k] += cost


def build_program(reps=1, ablate=()):
    nc = bacc.Bacc("TRN2", target_bir_lowering=False)
    blobs, lay = _host_consts()

    f_in = nc.dram_tensor("f_in", [B_PER_CORE, C1, 64, 64], BF16, kind="ExternalInput")
    k_in = nc.dram_tensor("k_all", [128, 2048], BF16, kind="ExternalInput")
    cb_ins = [nc.dram_tensor(f"cb{i}", list(b.shape), BF16, kind="ExternalInput")
              for i, b in enumerate(blobs)]
    # transposed output layout [b, X, ch, Y]; host returns .transpose(0,2,1,3)
    out_sh = nc.dram_tensor("out_sh", [B_PER_CORE, 128, NCH_OUT, 128], F32,
                            kind="ExternalOutput")

    import os
    wv = float(os.environ.get("KWV", "1.6"))   # DVE weight for prod/sub split
    wp = float(os.environ.get("KWP", "1.0"))   # Pool weight

    with tile.TileContext(nc) as tc:
        with (
            tc.tile_pool(name="cp", bufs=1) as cp,
            tc.tile_pool(name="fld", bufs=2) as fld,     # per-sample u/v/out
            tc.tile_pool(name="wk", bufs=2) as wk,       # small working tiles
            tc.tile_pool(name="mw", bufs=2) as mwp,      # conv wide tiles
            tc.tile_pool(name="wp", bufs=2) as wpp,      # cross product blocks
            tc.tile_pool(name="pp", bufs=1, space="PSUM") as pp,
        ):
            # ---- load constants: CB1 first (gates stage1), then k, CB2 ----
            CBs = [cp.tile(list(b.shape), BF16, tag=f"CB{i}", name=f"CB{i}")
                   for i, b in enumerate(blobs)]
            nc.sync.dma_start(out=CBs[0][:], in_=cb_ins[0][:])
            k_sb = cp.tile([128, 2048], BF16, tag="k_sb", name="k_sb")
            nc.sync.dma_start(out=k_sb[:], in_=k_in[:])
            nc.sync.dma_start(out=CBs[1][:], in_=cb_ins[1][:])

            def cview(name):
                bi, r, w, o = lay[name]
                return CBs[bi][0:r, o:o + w]

            cExFT1 = cview("ExFT1")
            cEyCT = cview("EyCT")
            cEyST = cview("EyST")
            cS_sel = cview("S_sel")
            ctsg = cview("tsg")
            cQF1 = cview("QF1")
            cQF2 = cview("QF2")
            cPRT = cview("PRT")
            cnPIT = cview("nPIT")
            cCyT = cview("CyT")

            # global greedy balancer across DVE / ACT / Pool
    
            es = EngSched({'v': nc.vector, 'a': nc.scalar, 'p': nc.gpsimd})

            def cp_copy(out, in_, free):
                # PSUM->SBUF copy; cost model: ACT 0.833/el + bubble,
                # DVE 1.042/el + bubble, Pool 0.833/el (no errata bubble)
                eng = es.pick({'a': free * 0.833 + 190,
                               'v': free * 1.042 + 180,
                               'p': free * 0.833 + 80})
                if eng is nc.scalar:
                    eng.copy(out=out, in_=in_)
                else:
                    eng.tensor_copy(out, in_)

            st = {b: {} for b in range(B_PER_CORE)}

            def stage1(b):
                s = st[b]
                s['OUT'] = fld.tile([128, NCH_OUT * 128], BF16, tag="OUT",
                                    name="OUT")
                fsb = wk.tile([64, 512], BF16, tag="fsb", name="fsb")
                nc.sync.dma_start(
                    out=fsb[:].rearrange("x (i y) -> x i y", i=8),
                    in_=f_in[b].rearrange("i x y -> x i y"))
                A_sb = []
                for ip in range(4):
                    psA = pp.tile([128, 320], F32, tag="bankA", bufs=2, name="psA")
                    nc.tensor.matmul(psA[:], fsb[:, ip * 128:(ip + 1) * 128],
                                     cExFT1, start=True, stop=True)
                    a_t = wk.tile([128, 320], BF16, tag=f"a{ip}", name="a_t")
                    cp_copy(a_t[:], psA[:], 320)
                    A_sb.append(a_t)
                s['A_sb'] = A_sb

            def stage_fr(b):
                s = st[b]
                A_sb, OUT = s['A_sb'], s['OUT']
                for iph in range(4):   # 2 channels per psUf tile
                    psUf = pp.tile([128, 256], F32, tag="bankA", bufs=2, name="psUf")
                    for iloc2 in range(2):
                        i = 2 * iph + iloc2
                        ip, iloc = i // 2, i % 2
                        t1 = A_sb[ip][iloc * 64:(iloc + 1) * 64, 192:320]
                        nc.tensor.matmul(psUf[:, iloc2 * 128:(iloc2 + 1) * 128],
                                         t1, cCyT[iloc * 64:(iloc + 1) * 64, :],
                                         start=True, stop=True)
                    cp_copy(OUT[:, iph * 256:(iph + 1) * 256], psUf[:], 256)
                if 'dma' not in ablate:
                    nc.gpsimd.dma_start(
                        out=out_sh[b, :, 0:8, :],
                        in_=OUT[:, 0:1024].rearrange("x (c y) -> x c y", c=8))

            def stage2(b):
                s = st[b]
                A_sb = s['A_sb']
                psFcv = [pp.tile([128, 128], F32, tag=f"bankF{4 + h}", name="psFcv")
                         for h in range(2)]
                for i in range(8):
                    iloc = i % 2
                    ysl = slice(iloc * 64, (iloc + 1) * 64)
                    A_RI = A_sb[i // 2][ysl, 0:128]
                    A_IS = A_sb[i // 2][ysl, 64:192]
                    h, im = i // 4, i % 4
                    sl = slice(im * 32, (im + 1) * 32)
                    tp = (0, im * 32)
                    nc.tensor.matmul(psFcv[h][sl, :], cEyCT[ysl, :], A_RI,
                                     start=True, stop=False, tile_position=tp)
                    nc.tensor.matmul(psFcv[h][sl, :], cEyST[ysl, :], A_IS,
                                     start=False, stop=True, tile_position=tp)
                Fcv = wk.tile([128, 256], BF16, tag="Fcv", name="Fcv")
                for h in range(2):
                    cp_copy(_view(Fcv[:], h * 64,
                                  [Fcv[:].ap[0], [128, 2], [1, 64]]),
                            psFcv[h][:].rearrange("p (r k) -> p r k", r=2), 128)
                s['Fcv'] = Fcv

            def stage_conv(b):
                s = st[b]
                Fcv = s['Fcv']
                Mw = []
                for RI in range(2):
                    m_t = mwp.tile([128, 2048], BF16, tag=f"mw{RI}", name="m_t")
                    conv_eng = es.pick({'v': 2048 * 0.52 + 60,
                                        'p': 2048 * 0.833 + 60})
                    conv_eng.tensor_mul(
                        m_t[:].rearrange("p (j f) -> p j f", j=16),
                        _bcast(Fcv[:, RI * 128:(RI + 1) * 128], 16),
                        k_sb[:].rearrange("p (j f) -> p j f", j=16))
                    Mw.append(m_t)

                acv_sb = wk.tile([32, 2048], BF16, tag="acv", name="acv_sb")
                for RI in range(2):
                    for jh in range(2):
                        ps_acv = pp.tile([32, 512], F32, tag="bankA", bufs=2,
                                         name="ps_acv")
                        for h in range(2):
                            rhs = _view(Mw[RI][:], jh * 1024 + h * 64,
                                        [Mw[RI][:].ap[0], [128, 8], [1, 64]])
                            nc.tensor.matmul(ps_acv[:], cS_sel, rhs,
                                             start=(h == 0), stop=(h == 1))
                        cp_copy(
                            acv_sb[:, (RI * 2 + jh) * 512:(RI * 2 + jh + 1) * 512],
                            ps_acv[:], 512)
                s['acv_sb'] = acv_sb

            def stage_B(b):
                s = st[b]
                acv_sb = s['acv_sb']
                # ---------------- uncurl: B = acv (*) t/s ----------------
                BuR = wk.tile([32, 1024], BF16, tag="BuR", name="BuR")
                BuI = wk.tile([32, 1024], BF16, tag="BuI", name="BuI")
                BvR = wk.tile([32, 1024], BF16, tag="BvR", name="BvR")
                BvI = wk.tile([32, 1024], BF16, tag="BvI", name="BvI")
                # acv layout: [R jh0 | R jh1 | I jh0 | I jh1] each 512
                # tsg: [-t | t | -s | s] each 512 (j-repeated, j-independent)
                for RI in range(2):
                    a_v = _view(acv_sb[:], RI * 1024,
                                [acv_sb[:].ap[0], [512, 2], [1, 512]])
                    bc = {'v': 1024 * 0.52 + 60, 'p': 1024 * 0.833 + 60}
                    if RI == 0:  # A_R -> imag parts (mult by +t / +s)
                        es.pick(bc).tensor_mul(
                            BuI[:].rearrange("p (j f) -> p j f", j=2),
                            a_v, _bcast(ctsg[:, 512:1024], 2))
                        es.pick(bc).tensor_mul(
                            BvI[:].rearrange("p (j f) -> p j f", j=2),
                            a_v, _bcast(ctsg[:, 1536:2048], 2))
                    else:        # A_I -> real parts (mult by -t / -s)
                        es.pick(bc).tensor_mul(
                            BuR[:].rearrange("p (j f) -> p j f", j=2),
                            a_v, _bcast(ctsg[:, 0:512], 2))
                        es.pick(bc).tensor_mul(
                            BvR[:].rearrange("p (j f) -> p j f", j=2),
                            a_v, _bcast(ctsg[:, 1024:1536], 2))
                s['B'] = (BuR, BuI, BvR, BvI)
                s['u_all'] = fld.tile([128, 2048], BF16, tag="u_all", name="u_all")
                s['v_all'] = fld.tile([128, 2048], BF16, tag="v_all", name="v_all")

            tog_ctr = [0]

            def synth_group(b, field, g2):
                """4 channels (2 cpairs) -> dest[:, g2*512:(g2+1)*512]."""
                s = st[b]
                BuR, BuI, BvR, BvI = s['B']
                BR, BI = (BuR, BuI) if field == 'u' else (BvR, BvI)
                dest = s['u_all'] if field == 'u' else s['v_all']
                tog = tog_ctr[0]
                tog_ctr[0] += 1
                psG = pp.tile([128, 512], F32, tag=f"bankF{tog % 2}",
                              name="psG")
                for sub in range(2):
                    cpair = 2 * g2 + sub
                    csl = slice(cpair * 128, (cpair + 1) * 128)
                    osl = slice(sub * 256, (sub + 1) * 256)
                    nc.tensor.matmul(psG[:, osl], BR[:, csl], cQF1,
                                     start=True, stop=False)
                    nc.tensor.matmul(psG[:, osl], BI[:, csl], cQF2,
                                     start=False, stop=True)
                G_sb = wk.tile([128, 512], BF16, tag="G_sb", name="G_sb")
                cp_copy(G_sb[:], psG[:], 512)
                psU = pp.tile([128, 512], F32, tag=f"bankF{2 + tog % 2}",
                              name="psU")
                for chl in range(4):
                    sub, chp = chl // 2, chl % 2
                    gr = G_sb[chp * 64:(chp + 1) * 64,
                              sub * 256:sub * 256 + 128]
                    gi = G_sb[chp * 64:(chp + 1) * 64,
                              sub * 256 + 128:sub * 256 + 256]
                    psl = slice(chp * 64, (chp + 1) * 64)
                    osl = slice(chl * 128, (chl + 1) * 128)
                    nc.tensor.matmul(psU[:, osl], cPRT[psl, :], gr,
                                     start=True, stop=False)
                    nc.tensor.matmul(psU[:, osl], cnPIT[psl, :], gi,
                                     start=False, stop=True)
                cp_copy(dest[:, g2 * 512:(g2 + 1) * 512], psU[:], 512)

            def emit_cross_block(b, gI, gJ):
                u_all, v_all, OUT = st[b]['u_all'], st[b]['v_all'], st[b]['OUT']
                W1 = wpp.tile([128, 2048], BF16, tag="W1", name="W1")
                # one 4D-AP product op per W tile: [p, a(4), b(4), 128]
                in0 = _view(u_all[:], gI * 512,
                            [u_all[:].ap[0], [128, 4], [0, 4], [1, 128]])
                in1 = _view(v_all[:], gJ * 512,
                            [v_all[:].ap[0], [0, 4], [128, 4], [1, 128]])
                es.pick({'v': 2048 * 0.52 + 60,
                         'p': 2048 * 0.833 + 60}).tensor_mul(
                    W1[:].rearrange("p (a cb f) -> p a cb f", a=4, cb=4),
                    in0, in1)
                if gI != gJ:
                    W2 = wpp.tile([128, 2048], BF16, tag="W2", name="W2")
                    in0 = _view(u_all[:], gJ * 512,
                                [u_all[:].ap[0], [128, 4], [0, 4], [1, 128]])
                    in1 = _view(v_all[:], gI * 512,
                                [v_all[:].ap[0], [0, 4], [128, 4], [1, 128]])
                    es.pick({'v': 2048 * 0.52 + 60,
                             'p': 2048 * 0.833 + 60}).tensor_mul(
                        W2[:].rearrange("p (bj ca f) -> p bj ca f", bj=4, ca=4),
                        in0, in1)
                    for ai in range(4):
                        a = 4 * gI + ai
                        pch = 8 + _PAIR_IDX[(a, 4 * gJ)]
                        in0 = W1[:, ai * 512:(ai + 1) * 512].rearrange(
                            "p (cb f) -> p cb f", cb=4)
                        in1 = _view(W2[:], ai * 128,
                                    [W2[:].ap[0], [512, 4], [1, 128]])
                        out = _view(OUT[:], pch * 128,
                                    [OUT[:].ap[0], [128, 4], [1, 128]])
                        es.pick({'v': 512 * 0.52 + 60,
                                 'p': 512 * 0.833 + 60}).tensor_sub(out, in0, in1)
                else:
                    for ai in range(3):
                        a = 4 * gI + ai
                        cnt = 3 - ai
                        pch = 8 + _PAIR_IDX[(a, a + 1)]
                        in0 = _view(W1[:], ai * 512 + (ai + 1) * 128,
                                    [W1[:].ap[0], [128, cnt], [1, 128]])
                        in1 = _view(W1[:], (ai + 1) * 512 + ai * 128,
                                    [W1[:].ap[0], [512, cnt], [1, 128]])
                        out = _view(OUT[:], pch * 128,
                                    [OUT[:].ap[0], [128, cnt], [1, 128]])
                        es.pick({'v': cnt * 128 * 0.52 + 60,
                                 'p': cnt * 128 * 0.833 + 60}).tensor_sub(
                            out, in0, in1)

            def cross_dma(b, c0, c1):
                OUT = st[b]['OUT']
                es.charge('p', 1100.0)
                nc.gpsimd.dma_start(
                    out=out_sh[b, :, c0:c1, :],
                    in_=OUT[:, c0 * 128:c1 * 128].rearrange(
                        "x (c y) -> x c y", c=c1 - c0))

            # ---------------- staggered emission across samples ----------
            # b0 runs ~half a pipeline ahead of b1 so b1's PE/ACT front work
            # overlaps b0's DVE/Pool cross work and the out-DMA stream.
            chunk_hi = [62, 100, 122, 128]

            def front(b):
                stage1(b)
                stage_fr(b)
                stage2(b)
                stage_conv(b)
                stage_B(b)

            def dma_row(b, gI):
                if 'dma' in ablate:
                    return
                c0 = 8 if gI == 0 else chunk_hi[gI - 1]
                cross_dma(b, c0, chunk_hi[gI])

            front(0)
            synth_group(0, 'v', 0)
            synth_group(0, 'u', 0)
            emit_cross_block(0, 0, 0)
            front(1)
            for gJ in range(1, 4):
                synth_group(0, 'v', gJ)
                emit_cross_block(0, 0, gJ)
            dma_row(0, 0)
            synth_group(1, 'v', 0)
            synth_group(1, 'u', 0)
            emit_cross_block(1, 0, 0)
            synth_group(0, 'u', 1)
            for gJ in range(1, 4):
                emit_cross_block(0, 1, gJ)
            dma_row(0, 1)
            synth_group(1, 'v', 1)
            emit_cross_block(1, 0, 1)
            synth_group(0, 'u', 2)
            for gJ in range(2, 4):
                emit_cross_block(0, 2, gJ)
            dma_row(0, 2)
            synth_group(1, 'v', 2)
            emit_cross_block(1, 0, 2)
            synth_group(0, 'u', 3)
            emit_cross_block(0, 3, 3)
            dma_row(0, 3)
            synth_group(1, 'v', 3)
            emit_cross_block(1, 0, 3)
            dma_row(1, 0)
            synth_group(1, 'u', 1)
            for gJ in range(1, 4):
                emit_cross_block(1, 1, gJ)
            dma_row(1, 1)
            synth_group(1, 'u', 2)
            for gJ in range(2, 4):
                emit_cross_block(1, 2, gJ)
            dma_row(1, 2)
            synth_group(1, 'u', 3)
            emit_cross_block(1, 3, 3)
            dma_row(1, 3)
    nc.compile()
    return nc


# ---------------------------------------------------------------------------
# entry point
# ---------------------------------------------------------------------------

_PROGRAM = {}


def _get_program(reps=1, ablate=(), **kw):
    key = (reps, tuple(sorted(ablate)), tuple(sorted(kw.items())))
    if key not in _PROGRAM:
        _PROGRAM[key] = build_program(reps, ablate=ablate, **kw)
    return _PROGRAM[key]


LAST_EXEC_NS = None
LAST_RESULT = None


def kernel(f, kernel):
    global LAST_EXEC_NS, LAST_RESULT
    f_bf = np.ascontiguousarray(np.asarray(f), dtype=np.float32).astype(NPBF16)
    k_all = _prep_k_all(np.asarray(kernel))
    blob, _ = _host_consts()
    nc = _get_program()
    in_maps = [
        {"f_in": f_bf[2 * c:2 * c + 2], "k_all": k_all, "cb": blob}
        for c in range(N_CORES)
    ]
    import os
    trace = bool(os.environ.get("KERNEL_TRACE"))
    res = run_bass_kernel_spmd(nc, in_maps, list(range(N_CORES)), trace=trace)
    LAST_RESULT = res
    if res.exec_time_ns is not None:
        LAST_EXEC_NS = res.exec_time_ns
    out = np.concatenate([res.results[c]["out_sh"] for c in range(N_CORES)], axis=0)
    # device layout is [b, X, ch, Y]; return the [b, ch, X, Y] view
    return out.transpose(0, 2, 1, 3)


# revision 23
# speedup vs baseline: 1.4360x; 1.2497x over previous
"""Trainium2 Bass kernel for nn_EquivariantLayer (spectral equivariant layer).

Data-parallel over batch: 2 samples/core x 8 cores. All-bf16 pipeline:

  stage1:  psA = f^T @ [ExR^T|ExI^T|-ExR^T | Rx^T]   (one fused matmul/2ch)
  stage2:  F = Ey @ A       (c-major conv layout via tile_position packing)
  conv:    M = F (*) K elementwise (K real); i-reduction via selector matmul
  uncurl:  pure-imaginary TO_U/TO_V -> real mults by t/s tables
  synth:   per channel pair: G = B @ QF (Q-side), field = P @ G (P-side)
  fr:      direct 2x Fourier upsample fr_i = Rx @ f_i @ Cy^T
  cross:   u_a v_b - u_b v_a in bf16 on DVE/Pool, written straight into a
           per-sample output tile; SWDGE (gpsimd) DMAs cast bf16->f32 on the
           way out to HBM.

All matmul operands are bf16 (1 cycle/row on PE vs 4 for fp32); every
PSUM->SBUF copy casts f32 accumulators down to bf16. Output 16.8MB f32 per
core dominates DMA time; compute is sized to hide beneath it.
"""
import sys
import numpy as np
import ml_dtypes

if '/opt/trn_rl_repo' not in sys.path:
    sys.path.insert(0, '/opt/trn_rl_repo')

import concourse.bass as bass
from concourse import bacc
import concourse.mybir as mybir
import concourse.tile as tile
from concourse.bass import AP
from concourse.bass_utils import run_bass_kernel_spmd

F32 = mybir.dt.float32
BF16 = mybir.dt.bfloat16
NPBF16 = ml_dtypes.bfloat16
N_CORES = 8
B_PER_CORE = 2
C1, C2, N1, N2 = 8, 16, 64, 128
NCH_OUT = 128  # 8 fr + 120 cross

I_IDX, J_IDX = np.triu_indices(C2, 1)
_PAIR_IDX = {}
for _p, (_a, _b) in enumerate(zip(I_IDX, J_IDX)):
    _PAIR_IDX[(int(_a), int(_b))] = _p


# ---------------------------------------------------------------------------
# host-side constant construction
# ---------------------------------------------------------------------------

def _host_consts():
    x = np.arange(64)
    kx = np.arange(64)
    c = np.arange(32)
    y = np.arange(64)
    X = np.arange(128)
    Y = np.arange(128)

    FRs = np.where(kx <= 32, kx, kx - 64).astype(np.float64)  # signed row freq

    ExR = np.cos(2 * np.pi * np.outer(kx, x) / 64)   # [kx, x]
    ExI = -np.sin(2 * np.pi * np.outer(kx, x) / 64)
    ExF = np.concatenate([ExR.T, ExI.T, -ExR.T], axis=1)   # [x, 192]

    EyCT = np.cos(2 * np.pi * np.outer(c, y) / 64).T   # [y=64, c=32]
    EyST = np.sin(2 * np.pi * np.outer(c, y) / 64).T
    EyCT2 = np.concatenate([EyCT, EyCT], axis=0)       # [128, 32] doubled rows
    EyST2 = np.concatenate([EyST, EyST], axis=0)

    S_sel = np.zeros((128, 32))
    for im in range(4):
        S_sel[im * 32 + np.arange(32), np.arange(32)] = 1.0

    den = FRs[None, :] ** 2 + c[:, None].astype(np.float64) ** 2
    den[0, 0] = 1.0
    t_u = c[:, None] / den                           # [32, 64]
    s_v = -FRs[None, :] / den
    t_rep = np.tile(t_u, (1, 8))                     # [32, 512] (j-rep)
    s_rep = np.tile(s_v, (1, 8))
    tsg = np.concatenate([-t_rep, t_rep, -s_rep, s_rep], axis=1)  # [32, 2048]

    w_c = np.where(c == 0, 1.0, 2.0)
    s_q = 2.0 / (128.0 * 128.0)
    QRT = (s_q * w_c[None, :] * np.cos(2 * np.pi * np.outer(Y, c) / 128)).T  # [c, Y]
    QIT = (s_q * w_c[None, :] * np.sin(2 * np.pi * np.outer(Y, c) / 128)).T
    QF1 = np.concatenate([QRT, QIT], axis=1)         # [32, 256]
    QF2 = np.concatenate([-QIT, QRT], axis=1)

    PRT = np.cos(2 * np.pi * np.outer(FRs, X) / 128)   # [kx=64, X=128]
    PIT = np.sin(2 * np.pi * np.outer(FRs, X) / 128)
    PRT[32, :] = 0.0
    PIT[32, :] = 0.0
    # doubled rows so lhsT slices can match rhs base partition 0 or 64
    PRT2 = np.concatenate([PRT, PRT], axis=0)          # [128, 128]
    nPIT2 = np.concatenate([-PIT, -PIT], axis=0)

    # direct fr path: fr_i = Rx @ f_i @ Cy^T (pure 2x Fourier upsampling)
    ExRm = np.cos(2 * np.pi * np.outer(kx, x) / 64)
    ExIm = -np.sin(2 * np.pi * np.outer(kx, x) / 64)
    EyRm = np.cos(2 * np.pi * np.outer(c, y) / 64)
    EyIm = -np.sin(2 * np.pi * np.outer(c, y) / 64)
    QRm = s_q * w_c[None, :] * np.cos(2 * np.pi * np.outer(Y, c) / 128)
    QIm = s_q * w_c[None, :] * np.sin(2 * np.pi * np.outer(Y, c) / 128)
    Rx = PRT.T @ ExRm - PIT.T @ ExIm                 # [128, 64]
    Cy = QRm @ EyRm - QIm @ EyIm                     # [128, 64]
    RxT = Rx.T                                       # [x=64, X=128]
    CyT = np.concatenate([Cy.T, Cy.T], axis=0)       # [128, 128] doubled rows

    ExFT1 = np.concatenate([ExF, RxT], axis=1)       # [64, 320]

    # pack consts into two [128, W] bf16 blobs: CB1 = front-stage consts
    # (small, loads fast so stage1 starts early), CB2 = the rest.
    blobs = []
    layout = {}
    for bi, consts in enumerate([
        dict(ExFT1=ExFT1, EyCT=EyCT2, EyST=EyST2, S_sel=S_sel, CyT=CyT),
        dict(tsg=tsg, QF1=QF1, QF2=QF2, PRT=PRT2, nPIT=nPIT2),
    ]):
        off = 0
        for name, arr in consts.items():
            layout[name] = (bi, arr.shape[0], arr.shape[1], off)
            off += arr.shape[1]
        blob = np.zeros((128, off), dtype=NPBF16)
        for name, arr in consts.items():
            _, r, w, o = layout[name]
            blob[:r, o:o + w] = arr.astype(NPBF16)
        blobs.append(blob)
    return blobs, layout


def _rot90_kernel(k):
    y = np.swapaxes(k, -2, -1)
    return np.concatenate([y[..., :1], y[..., :0:-1]], axis=-1)


def _symmetric_kernel(k):
    k1 = k
    k2 = _rot90_kernel(k1)
    k3 = _rot90_kernel(k2)
    k4 = _rot90_kernel(k3)
    k5 = np.swapaxes(k1, -2, -1)
    k6 = _rot90_kernel(k5)
    k7 = _rot90_kernel(k6)
    k8 = _rot90_kernel(k7)
    return (k1 + k2 + k3 + k4 + k5 + k6 + k7 + k8) / 8.0


def _prep_k_all(kernel_np):
    """kernel [1,8,16,64,64] -> k_all [128, 2048] conv-layout packed (bf16)."""
    ksym = _symmetric_kernel(kernel_np.astype(np.float64))[0]   # [8,16,64,64]
    K = np.fft.rfft2(ksym).real                                  # [8,16,64,33]
    Kc = np.transpose(K[:, :, :, :32], (0, 1, 3, 2)).copy()      # [i,j,c,kx]
    Kc[:, :, :, 32] = 0.0                                        # kx nyquist
    k_all = np.zeros((128, 2048), dtype=NPBF16)
    for i in range(8):
        h, im = i // 4, i % 4
        for j in range(16):
            k_all[im * 32:(im + 1) * 32, j * 128 + h * 64: j * 128 + h * 64 + 64] = \
                Kc[i, j].astype(NPBF16)
    return k_all


# ---------------------------------------------------------------------------
# device program
# ---------------------------------------------------------------------------

def _bcast(ap, n, axis_pos=1):
    """Insert a zero-step broadcast dim of size n into an AP."""
    dims = list(ap.ap)
    dims.insert(axis_pos, [0, n])
    return AP(ap.tensor, ap.offset, dims)


def _view(ap, offset_elems, dims):
    """Raw AP view on the same tensor: explicit offset (elems) + [step, count]."""
    return AP(ap.tensor, ap.offset + offset_elems, dims)


class EngSched:
    """Greedy engine load balancer: pick the engine minimizing accumulated
    busy-ns + this op's cost on that engine."""

    def __init__(self, engmap):
        self.eng = engmap
        self.acc = {k: 0.0 for k in engmap}

    def pick(self, costs):
        k = min(costs, key=lambda k: self.acc[k] + costs[k])
        self.acc[k] += costs[k]
        return self.eng[k]

    def charge(self, k, cost):
        self.acc[k] += cost


def build_program(reps=1, ablate=()):
    nc = bacc.Bacc("TRN2", target_bir_lowering=False)
    blobs, lay = _host_consts()

    f_in = nc.dram_tensor("f_in", [B_PER_CORE, C1, 64, 64], BF16, kind="ExternalInput")
    k_in = nc.dram_tensor("k_all", [128, 2048], BF16, kind="ExternalInput")
    cb_ins = [nc.dram_tensor(f"cb{i}", list(b.shape), BF16, kind="ExternalInput")
              for i, b in enumerate(blobs)]
    # transposed output layout [b, X, ch, Y]; host returns .transpose(0,2,1,3)
    out_sh = nc.dram_tensor("out_sh", [B_PER_CORE, 128, NCH_OUT, 128], F32,
                            kind="ExternalOutput")

    import os
    wv = float(os.environ.get("KWV", "1.6"))   # DVE weight for prod/sub split
    wp = float(os.environ.get("KWP", "1.0"))   # Pool weight

    with tile.TileContext(nc) as tc:
        with (
            tc.tile_pool(name="cp", bufs=1) as cp,
            tc.tile_pool(name="fld", bufs=2) as fld,     # per-sample u/v/out
            tc.tile_pool(name="wk", bufs=2) as wk,       # small working tiles
            tc.tile_pool(name="mw", bufs=1) as mwp,      # conv wide tiles
            tc.tile_pool(name="wp", bufs=2) as wpp,      # cross product blocks
            tc.tile_pool(name="pp", bufs=1, space="PSUM") as pp,
        ):
            # ---- load constants: CB1 first (gates stage1), then k, CB2 ----
            CBs = [cp.tile(list(b.shape), BF16, tag=f"CB{i}", name=f"CB{i}")
                   for i, b in enumerate(blobs)]
            nc.sync.dma_start(out=CBs[0][:], in_=cb_ins[0][:])
            k_sb = cp.tile([128, 2048], BF16, tag="k_sb", name="k_sb")
            nc.sync.dma_start(out=k_sb[:], in_=k_in[:])
            nc.sync.dma_start(out=CBs[1][:], in_=cb_ins[1][:])

            def cview(name):
                bi, r, w, o = lay[name]
                return CBs[bi][0:r, o:o + w]

            cExFT1 = cview("ExFT1")
            cEyCT = cview("EyCT")
            cEyST = cview("EyST")
            cS_sel = cview("S_sel")
            ctsg = cview("tsg")
            cQF1 = cview("QF1")
            cQF2 = cview("QF2")
            cPRT = cview("PRT")
            cnPIT = cview("nPIT")
            cCyT = cview("CyT")

            # global greedy balancer across DVE / ACT / Pool
    
            es = EngSched({'v': nc.vector, 'a': nc.scalar, 'p': nc.gpsimd})

            def cp_copy(out, in_, free):
                # PSUM->SBUF copy; cost model: ACT 0.833/el + bubble,
                # DVE 1.042/el + bubble, Pool 0.833/el (no errata bubble)
                eng = es.pick({'a': free * 0.833 + 190,
                               'v': free * 1.042 + 180,
                               'p': free * 0.833 + 80})
                if eng is nc.scalar:
                    eng.copy(out=out, in_=in_)
                else:
                    eng.tensor_copy(out, in_)

            st = {b: {} for b in range(B_PER_CORE)}

            def stage1(b):
                s = st[b]
                s['OUT'] = fld.tile([128, NCH_OUT * 128], F32, tag="OUT",
                                    name="OUT")
                fsb = wk.tile([64, 512], BF16, tag="fsb", name="fsb", bufs=1)
                nc.sync.dma_start(
                    out=fsb[:].rearrange("x (i y) -> x i y", i=8),
                    in_=f_in[b].rearrange("i x y -> x i y"))
                A_sb = []
                for ip in range(4):
                    psA = pp.tile([128, 320], F32, tag="bankA", bufs=2, name="psA")
                    nc.tensor.matmul(psA[:], fsb[:, ip * 128:(ip + 1) * 128],
                                     cExFT1, start=True, stop=True)
                    a_t = wk.tile([128, 320], BF16, tag=f"a{ip}", name="a_t",
                                  bufs=1)
                    cp_copy(a_t[:], psA[:], 320)
                    A_sb.append(a_t)
                s['A_sb'] = A_sb

            def stage_fr(b):
                s = st[b]
                A_sb, OUT = s['A_sb'], s['OUT']
                for iph in range(4):   # 2 channels per psUf tile
                    psUf = pp.tile([128, 256], F32, tag="bankA", bufs=2, name="psUf")
                    for iloc2 in range(2):
                        i = 2 * iph + iloc2
                        ip, iloc = i // 2, i % 2
                        t1 = A_sb[ip][iloc * 64:(iloc + 1) * 64, 192:320]
                        nc.tensor.matmul(psUf[:, iloc2 * 128:(iloc2 + 1) * 128],
                                         t1, cCyT[iloc * 64:(iloc + 1) * 64, :],
                                         start=True, stop=True)
                    cp_copy(OUT[:, iph * 256:(iph + 1) * 256], psUf[:], 256)
                if 'dma' not in ablate:
                    nc.sync.dma_start(
                        out=out_sh[b, :, 0:8, :],
                        in_=OUT[:, 0:1024].rearrange("x (c y) -> x c y", c=8))

            def stage2(b):
                s = st[b]
                A_sb = s['A_sb']
                psFcv = [pp.tile([128, 128], F32, tag=f"bankF{4 + h}", name="psFcv")
                         for h in range(2)]
                for i in range(8):
                    iloc = i % 2
                    ysl = slice(iloc * 64, (iloc + 1) * 64)
                    A_RI = A_sb[i // 2][ysl, 0:128]
                    A_IS = A_sb[i // 2][ysl, 64:192]
                    h, im = i // 4, i % 4
                    sl = slice(im * 32, (im + 1) * 32)
                    tp = (0, im * 32)
                    nc.tensor.matmul(psFcv[h][sl, :], cEyCT[ysl, :], A_RI,
                                     start=True, stop=False, tile_position=tp)
                    nc.tensor.matmul(psFcv[h][sl, :], cEyST[ysl, :], A_IS,
                                     start=False, stop=True, tile_position=tp)
                Fcv = wk.tile([128, 256], BF16, tag="Fcv", name="Fcv")
                for h in range(2):
                    cp_copy(_view(Fcv[:], h * 64,
                                  [Fcv[:].ap[0], [128, 2], [1, 64]]),
                            psFcv[h][:].rearrange("p (r k) -> p r k", r=2), 128)
                s['Fcv'] = Fcv

            def stage_conv(b):
                s = st[b]
                Fcv = s['Fcv']
                Mw = []
                for RI in range(2):
                    m_t = mwp.tile([128, 2048], BF16, tag=f"mw{RI}", name="m_t")
                    conv_eng = es.pick({'v': 2048 * 0.52 + 60,
                                        'p': 2048 * 0.833 + 60})
                    conv_eng.tensor_mul(
                        m_t[:].rearrange("p (j f) -> p j f", j=16),
                        _bcast(Fcv[:, RI * 128:(RI + 1) * 128], 16),
                        k_sb[:].rearrange("p (j f) -> p j f", j=16))
                    Mw.append(m_t)

                acv_sb = wk.tile([32, 2048], BF16, tag="acv", name="acv_sb",
                                 bufs=1)
                for RI in range(2):
                    for jh in range(2):
                        ps_acv = pp.tile([32, 512], F32, tag="bankA", bufs=2,
                                         name="ps_acv")
                        for h in range(2):
                            rhs = _view(Mw[RI][:], jh * 1024 + h * 64,
                                        [Mw[RI][:].ap[0], [128, 8], [1, 64]])
                            nc.tensor.matmul(ps_acv[:], cS_sel, rhs,
                                             start=(h == 0), stop=(h == 1))
                        cp_copy(
                            acv_sb[:, (RI * 2 + jh) * 512:(RI * 2 + jh + 1) * 512],
                            ps_acv[:], 512)
                s['acv_sb'] = acv_sb

            def stage_B(b):
                s = st[b]
                acv_sb = s['acv_sb']
                # ---------------- uncurl: B = acv (*) t/s ----------------
                BuR = wk.tile([32, 1024], BF16, tag="BuR", name="BuR")
                BuI = wk.tile([32, 1024], BF16, tag="BuI", name="BuI")
                BvR = wk.tile([32, 1024], BF16, tag="BvR", name="BvR")
                BvI = wk.tile([32, 1024], BF16, tag="BvI", name="BvI")
                # acv layout: [R jh0 | R jh1 | I jh0 | I jh1] each 512
                # tsg: [-t | t | -s | s] each 512 (j-repeated, j-independent)
                for RI in range(2):
                    a_v = _view(acv_sb[:], RI * 1024,
                                [acv_sb[:].ap[0], [512, 2], [1, 512]])
                    bc = {'v': 1024 * 0.52 + 60, 'p': 1024 * 0.833 + 60}
                    if RI == 0:  # A_R -> imag parts (mult by +t / +s)
                        es.pick(bc).tensor_mul(
                            BuI[:].rearrange("p (j f) -> p j f", j=2),
                            a_v, _bcast(ctsg[:, 512:1024], 2))
                        es.pick(bc).tensor_mul(
                            BvI[:].rearrange("p (j f) -> p j f", j=2),
                            a_v, _bcast(ctsg[:, 1536:2048], 2))
                    else:        # A_I -> real parts (mult by -t / -s)
                        es.pick(bc).tensor_mul(
                            BuR[:].rearrange("p (j f) -> p j f", j=2),
                            a_v, _bcast(ctsg[:, 0:512], 2))
                        es.pick(bc).tensor_mul(
                            BvR[:].rearrange("p (j f) -> p j f", j=2),
                            a_v, _bcast(ctsg[:, 1024:1536], 2))
                s['B'] = (BuR, BuI, BvR, BvI)
                s['u_all'] = fld.tile([128, 2048], BF16, tag="u_all", name="u_all")
                s['v_all'] = fld.tile([128, 2048], BF16, tag="v_all", name="v_all")

            tog_ctr = [0]

            def synth_group(b, field, g2):
                """4 channels (2 cpairs) -> dest[:, g2*512:(g2+1)*512]."""
                s = st[b]
                BuR, BuI, BvR, BvI = s['B']
                BR, BI = (BuR, BuI) if field == 'u' else (BvR, BvI)
                dest = s['u_all'] if field == 'u' else s['v_all']
                tog = tog_ctr[0]
                tog_ctr[0] += 1
                psG = pp.tile([128, 512], F32, tag=f"bankF{tog % 2}",
                              name="psG")
                for sub in range(2):
                    cpair = 2 * g2 + sub
                    csl = slice(cpair * 128, (cpair + 1) * 128)
                    osl = slice(sub * 256, (sub + 1) * 256)
                    nc.tensor.matmul(psG[:, osl], BR[:, csl], cQF1,
                                     start=True, stop=False)
                    nc.tensor.matmul(psG[:, osl], BI[:, csl], cQF2,
                                     start=False, stop=True)
                G_sb = wk.tile([128, 512], BF16, tag="G_sb", name="G_sb")
                cp_copy(G_sb[:], psG[:], 512)
                psU = pp.tile([128, 512], F32, tag=f"bankF{2 + tog % 2}",
                              name="psU")
                for chl in range(4):
                    sub, chp = chl // 2, chl % 2
                    gr = G_sb[chp * 64:(chp + 1) * 64,
                              sub * 256:sub * 256 + 128]
                    gi = G_sb[chp * 64:(chp + 1) * 64,
                              sub * 256 + 128:sub * 256 + 256]
                    psl = slice(chp * 64, (chp + 1) * 64)
                    osl = slice(chl * 128, (chl + 1) * 128)
                    nc.tensor.matmul(psU[:, osl], cPRT[psl, :], gr,
                                     start=True, stop=False)
                    nc.tensor.matmul(psU[:, osl], cnPIT[psl, :], gi,
                                     start=False, stop=True)
                cp_copy(dest[:, g2 * 512:(g2 + 1) * 512], psU[:], 512)

            def emit_cross_block(b, gI, gJ):
                u_all, v_all, OUT = st[b]['u_all'], st[b]['v_all'], st[b]['OUT']
                W1 = wpp.tile([128, 2048], BF16, tag="W1", name="W1")
                # one 4D-AP product op per W tile: [p, a(4), b(4), 128]
                in0 = _view(u_all[:], gI * 512,
                            [u_all[:].ap[0], [128, 4], [0, 4], [1, 128]])
                in1 = _view(v_all[:], gJ * 512,
                            [v_all[:].ap[0], [0, 4], [128, 4], [1, 128]])
                es.pick({'v': 2048 * 0.52 + 60,
                         'p': 2048 * 0.833 + 60}).tensor_mul(
                    W1[:].rearrange("p (a cb f) -> p a cb f", a=4, cb=4),
                    in0, in1)
                if gI != gJ:
                    W2 = wpp.tile([128, 2048], BF16, tag="W2", name="W2")
                    in0 = _view(u_all[:], gJ * 512,
                                [u_all[:].ap[0], [128, 4], [0, 4], [1, 128]])
                    in1 = _view(v_all[:], gI * 512,
                                [v_all[:].ap[0], [0, 4], [128, 4], [1, 128]])
                    es.pick({'v': 2048 * 0.52 + 60,
                             'p': 2048 * 0.833 + 60}).tensor_mul(
                        W2[:].rearrange("p (bj ca f) -> p bj ca f", bj=4, ca=4),
                        in0, in1)
                    for ai in range(4):
                        a = 4 * gI + ai
                        pch = 8 + _PAIR_IDX[(a, 4 * gJ)]
                        in0 = W1[:, ai * 512:(ai + 1) * 512].rearrange(
                            "p (cb f) -> p cb f", cb=4)
                        in1 = _view(W2[:], ai * 128,
                                    [W2[:].ap[0], [512, 4], [1, 128]])
                        out = _view(OUT[:], pch * 128,
                                    [OUT[:].ap[0], [128, 4], [1, 128]])
                        es.pick({'v': 512 * 1.042 + 60,
                                 'p': 512 * 0.833 + 60}).tensor_sub(out, in0, in1)
                else:
                    for ai in range(3):
                        a = 4 * gI + ai
                        cnt = 3 - ai
                        pch = 8 + _PAIR_IDX[(a, a + 1)]
                        in0 = _view(W1[:], ai * 512 + (ai + 1) * 128,
                                    [W1[:].ap[0], [128, cnt], [1, 128]])
                        in1 = _view(W1[:], (ai + 1) * 512 + ai * 128,
                                    [W1[:].ap[0], [512, cnt], [1, 128]])
                        out = _view(OUT[:], pch * 128,
                                    [OUT[:].ap[0], [128, cnt], [1, 128]])
                        es.pick({'v': cnt * 128 * 1.042 + 60,
                                 'p': cnt * 128 * 0.833 + 60}).tensor_sub(
                            out, in0, in1)

            def cross_dma(b, c0, c1):
                OUT = st[b]['OUT']
                nc.sync.dma_start(
                    out=out_sh[b, :, c0:c1, :],
                    in_=OUT[:, c0 * 128:c1 * 128].rearrange(
                        "x (c y) -> x c y", c=c1 - c0))

            # ---------------- staggered emission across samples ----------
            # b0 runs ~half a pipeline ahead of b1 so b1's PE/ACT front work
            # overlaps b0's DVE/Pool cross work and the out-DMA stream.
            chunk_hi = [62, 100, 122, 128]

            def front(b):
                stage1(b)
                stage_fr(b)
                stage2(b)
                stage_conv(b)
                stage_B(b)

            def dma_row(b, gI):
                if 'dma' in ablate:
                    return
                c0 = 8 if gI == 0 else chunk_hi[gI - 1]
                cross_dma(b, c0, chunk_hi[gI])

            front(0)
            synth_group(0, 'v', 0)
            synth_group(0, 'u', 0)
            emit_cross_block(0, 0, 0)
            front(1)
            for gJ in range(1, 4):
                synth_group(0, 'v', gJ)
                emit_cross_block(0, 0, gJ)
            dma_row(0, 0)
            synth_group(1, 'v', 0)
            synth_group(1, 'u', 0)
            emit_cross_block(1, 0, 0)
            synth_group(0, 'u', 1)
            for gJ in range(1, 4):
                emit_cross_block(0, 1, gJ)
            dma_row(0, 1)
            synth_group(1, 'v', 1)
            emit_cross_block(1, 0, 1)
            synth_group(0, 'u', 2)
            for gJ in range(2, 4):
                emit_cross_block(0, 2, gJ)
            dma_row(0, 2)
            synth_group(1, 'v', 2)
            emit_cross_block(1, 0, 2)
            synth_group(0, 'u', 3)
            emit_cross_block(0, 3, 3)
            dma_row(0, 3)
            synth_group(1, 'v', 3)
            emit_cross_block(1, 0, 3)
            dma_row(1, 0)
            synth_group(1, 'u', 1)
            for gJ in range(1, 4):
                emit_cross_block(1, 1, gJ)
            dma_row(1, 1)
            synth_group(1, 'u', 2)
            for gJ in range(2, 4):
                emit_cross_block(1, 2, gJ)
            dma_row(1, 2)
            synth_group(1, 'u', 3)
            emit_cross_block(1, 3, 3)
            dma_row(1, 3)
    nc.compile()
    return nc


# ---------------------------------------------------------------------------
# entry point
# ---------------------------------------------------------------------------

_PROGRAM = {}


def _get_program(reps=1, ablate=(), **kw):
    key = (reps, tuple(sorted(ablate)), tuple(sorted(kw.items())))
    if key not in _PROGRAM:
        _PROGRAM[key] = build_program(reps, ablate=ablate, **kw)
    return _PROGRAM[key]


LAST_EXEC_NS = None
LAST_RESULT = None


def kernel(f, kernel):
    global LAST_EXEC_NS, LAST_RESULT
    f_bf = np.ascontiguousarray(np.asarray(f), dtype=np.float32).astype(NPBF16)
    k_all = _prep_k_all(np.asarray(kernel))
    blobs, _ = _host_consts()
    nc = _get_program()
    in_maps = [
        {"f_in": f_bf[2 * c:2 * c + 2], "k_all": k_all,
         "cb0": blobs[0], "cb1": blobs[1]}
        for c in range(N_CORES)
    ]
    import os
    trace = bool(os.environ.get("KERNEL_TRACE"))
    res = run_bass_kernel_spmd(nc, in_maps, list(range(N_CORES)), trace=trace)
    LAST_RESULT = res
    if res.exec_time_ns is not None:
        LAST_EXEC_NS = res.exec_time_ns
    out = np.concatenate([res.results[c]["out_sh"] for c in range(N_CORES)], axis=0)
    # device layout is [b, X, ch, Y]; return the [b, ch, X, Y] view
    return out.transpose(0, 2, 1, 3)


# revision 24
# speedup vs baseline: 1.5219x; 1.0598x over previous
"""Trainium2 Bass kernel for nn_EquivariantLayer (spectral equivariant layer).

Data-parallel over batch: 2 samples/core x 8 cores. All-bf16 pipeline:

  stage1:  psA = f^T @ [ExR^T|ExI^T|-ExR^T | Rx^T]   (one fused matmul/2ch)
  stage2:  F = Ey @ A       (c-major conv layout via tile_position packing)
  conv:    M = F (*) K elementwise (K real); i-reduction via selector matmul
  uncurl:  pure-imaginary TO_U/TO_V -> real mults by t/s tables
  synth:   per channel pair: G = B @ QF (Q-side), field = P @ G (P-side)
  fr:      direct 2x Fourier upsample fr_i = Rx @ f_i @ Cy^T
  cross:   u_a v_b - u_b v_a in bf16 on DVE/Pool, written straight into a
           per-sample output tile; SWDGE (gpsimd) DMAs cast bf16->f32 on the
           way out to HBM.

All matmul operands are bf16 (1 cycle/row on PE vs 4 for fp32); every
PSUM->SBUF copy casts f32 accumulators down to bf16. Output 16.8MB f32 per
core dominates DMA time; compute is sized to hide beneath it.
"""
import sys
import numpy as np
import ml_dtypes

if '/opt/trn_rl_repo' not in sys.path:
    sys.path.insert(0, '/opt/trn_rl_repo')

import concourse.bass as bass
from concourse import bacc
import concourse.mybir as mybir
import concourse.tile as tile
from concourse.bass import AP
from concourse.bass_utils import run_bass_kernel_spmd

F32 = mybir.dt.float32
BF16 = mybir.dt.bfloat16
NPBF16 = ml_dtypes.bfloat16
N_CORES = 8
B_PER_CORE = 2
C1, C2, N1, N2 = 8, 16, 64, 128
NCH_OUT = 128  # 8 fr + 120 cross

I_IDX, J_IDX = np.triu_indices(C2, 1)
_PAIR_IDX = {}
for _p, (_a, _b) in enumerate(zip(I_IDX, J_IDX)):
    _PAIR_IDX[(int(_a), int(_b))] = _p


# ---------------------------------------------------------------------------
# host-side constant construction
# ---------------------------------------------------------------------------

def _host_consts():
    x = np.arange(64)
    kx = np.arange(64)
    c = np.arange(32)
    y = np.arange(64)
    X = np.arange(128)
    Y = np.arange(128)

    FRs = np.where(kx <= 32, kx, kx - 64).astype(np.float64)  # signed row freq

    ExR = np.cos(2 * np.pi * np.outer(kx, x) / 64)   # [kx, x]
    ExI = -np.sin(2 * np.pi * np.outer(kx, x) / 64)
    ExF = np.concatenate([ExR.T, ExI.T, -ExR.T], axis=1)   # [x, 192]

    EyCT = np.cos(2 * np.pi * np.outer(c, y) / 64).T   # [y=64, c=32]
    EyST = np.sin(2 * np.pi * np.outer(c, y) / 64).T
    EyCT2 = np.concatenate([EyCT, EyCT], axis=0)       # [128, 32] doubled rows
    EyST2 = np.concatenate([EyST, EyST], axis=0)

    S_sel = np.zeros((128, 32))
    for im in range(4):
        S_sel[im * 32 + np.arange(32), np.arange(32)] = 1.0

    den = FRs[None, :] ** 2 + c[:, None].astype(np.float64) ** 2
    den[0, 0] = 1.0
    t_u = c[:, None] / den                           # [32, 64]
    s_v = -FRs[None, :] / den
    t_rep = np.tile(t_u, (1, 8))                     # [32, 512] (j-rep)
    s_rep = np.tile(s_v, (1, 8))
    tsg = np.concatenate([-t_rep, t_rep, -s_rep, s_rep], axis=1)  # [32, 2048]

    w_c = np.where(c == 0, 1.0, 2.0)
    s_q = 2.0 / (128.0 * 128.0)
    QRT = (s_q * w_c[None, :] * np.cos(2 * np.pi * np.outer(Y, c) / 128)).T  # [c, Y]
    QIT = (s_q * w_c[None, :] * np.sin(2 * np.pi * np.outer(Y, c) / 128)).T
    QF1 = np.concatenate([QRT, QIT], axis=1)         # [32, 256]
    QF2 = np.concatenate([-QIT, QRT], axis=1)

    PRT = np.cos(2 * np.pi * np.outer(FRs, X) / 128)   # [kx=64, X=128]
    PIT = np.sin(2 * np.pi * np.outer(FRs, X) / 128)
    PRT[32, :] = 0.0
    PIT[32, :] = 0.0
    # doubled rows so lhsT slices can match rhs base partition 0 or 64
    PRT2 = np.concatenate([PRT, PRT], axis=0)          # [128, 128]
    nPIT2 = np.concatenate([-PIT, -PIT], axis=0)

    # direct fr path: fr_i = Rx @ f_i @ Cy^T (pure 2x Fourier upsampling)
    ExRm = np.cos(2 * np.pi * np.outer(kx, x) / 64)
    ExIm = -np.sin(2 * np.pi * np.outer(kx, x) / 64)
    EyRm = np.cos(2 * np.pi * np.outer(c, y) / 64)
    EyIm = -np.sin(2 * np.pi * np.outer(c, y) / 64)
    QRm = s_q * w_c[None, :] * np.cos(2 * np.pi * np.outer(Y, c) / 128)
    QIm = s_q * w_c[None, :] * np.sin(2 * np.pi * np.outer(Y, c) / 128)
    Rx = PRT.T @ ExRm - PIT.T @ ExIm                 # [128, 64]
    Cy = QRm @ EyRm - QIm @ EyIm                     # [128, 64]
    RxT = Rx.T                                       # [x=64, X=128]
    CyT = np.concatenate([Cy.T, Cy.T], axis=0)       # [128, 128] doubled rows

    ExFT1 = np.concatenate([ExF, RxT], axis=1)       # [64, 320]

    # pack consts into two [128, W] bf16 blobs: CB1 = front-stage consts
    # (small, loads fast so stage1 starts early), CB2 = the rest.
    blobs = []
    layout = {}
    for bi, consts in enumerate([
        dict(ExFT1=ExFT1, EyCT=EyCT2, EyST=EyST2, S_sel=S_sel, CyT=CyT),
        dict(tsg=tsg, QF1=QF1, QF2=QF2, PRT=PRT2, nPIT=nPIT2),
    ]):
        off = 0
        for name, arr in consts.items():
            layout[name] = (bi, arr.shape[0], arr.shape[1], off)
            off += arr.shape[1]
        blob = np.zeros((128, off), dtype=NPBF16)
        for name, arr in consts.items():
            _, r, w, o = layout[name]
            blob[:r, o:o + w] = arr.astype(NPBF16)
        blobs.append(blob)
    return blobs, layout


def _rot90_kernel(k):
    y = np.swapaxes(k, -2, -1)
    return np.concatenate([y[..., :1], y[..., :0:-1]], axis=-1)


def _symmetric_kernel(k):
    k1 = k
    k2 = _rot90_kernel(k1)
    k3 = _rot90_kernel(k2)
    k4 = _rot90_kernel(k3)
    k5 = np.swapaxes(k1, -2, -1)
    k6 = _rot90_kernel(k5)
    k7 = _rot90_kernel(k6)
    k8 = _rot90_kernel(k7)
    return (k1 + k2 + k3 + k4 + k5 + k6 + k7 + k8) / 8.0


def _prep_k_all(kernel_np):
    """kernel [1,8,16,64,64] -> k_all [128, 2048] conv-layout packed (bf16)."""
    ksym = _symmetric_kernel(kernel_np.astype(np.float64))[0]   # [8,16,64,64]
    K = np.fft.rfft2(ksym).real                                  # [8,16,64,33]
    Kc = np.transpose(K[:, :, :, :32], (0, 1, 3, 2)).copy()      # [i,j,c,kx]
    Kc[:, :, :, 32] = 0.0                                        # kx nyquist
    k_all = np.zeros((128, 2048), dtype=NPBF16)
    for i in range(8):
        h, im = i // 4, i % 4
        for j in range(16):
            k_all[im * 32:(im + 1) * 32, j * 128 + h * 64: j * 128 + h * 64 + 64] = \
                Kc[i, j].astype(NPBF16)
    return k_all


# ---------------------------------------------------------------------------
# device program
# ---------------------------------------------------------------------------

def _bcast(ap, n, axis_pos=1):
    """Insert a zero-step broadcast dim of size n into an AP."""
    dims = list(ap.ap)
    dims.insert(axis_pos, [0, n])
    return AP(ap.tensor, ap.offset, dims)


def _view(ap, offset_elems, dims):
    """Raw AP view on the same tensor: explicit offset (elems) + [step, count]."""
    return AP(ap.tensor, ap.offset + offset_elems, dims)


class EngSched:
    """Greedy engine load balancer: pick the engine minimizing accumulated
    busy-ns + this op's cost on that engine."""

    def __init__(self, engmap):
        self.eng = engmap
        self.acc = {k: 0.0 for k in engmap}

    def pick(self, costs):
        k = min(costs, key=lambda k: self.acc[k] + costs[k])
        self.acc[k] += costs[k]
        return self.eng[k]

    def charge(self, k, cost):
        self.acc[k] += cost


def build_program(reps=1, ablate=()):
    nc = bacc.Bacc("TRN2", target_bir_lowering=False)
    blobs, lay = _host_consts()

    f_in = nc.dram_tensor("f_in", [B_PER_CORE, C1, 64, 64], BF16, kind="ExternalInput")
    k_in = nc.dram_tensor("k_all", [128, 2048], BF16, kind="ExternalInput")
    cb_ins = [nc.dram_tensor(f"cb{i}", list(b.shape), BF16, kind="ExternalInput")
              for i, b in enumerate(blobs)]
    # transposed output layout [b, X, ch, Y]; host returns .transpose(0,2,1,3)
    out_sh = nc.dram_tensor("out_sh", [B_PER_CORE, 128, NCH_OUT, 128], F32,
                            kind="ExternalOutput")

    import os
    wv = float(os.environ.get("KWV", "1.6"))   # DVE weight for prod/sub split
    wp = float(os.environ.get("KWP", "1.0"))   # Pool weight

    with tile.TileContext(nc) as tc:
        with (
            tc.tile_pool(name="cp", bufs=1) as cp,
            tc.tile_pool(name="fld", bufs=2) as fld,     # per-sample u/v/out
            tc.tile_pool(name="wk", bufs=2) as wk,       # small working tiles
            tc.tile_pool(name="mw", bufs=1) as mwp,      # conv wide tiles
            tc.tile_pool(name="wp", bufs=2) as wpp,      # cross product blocks
            tc.tile_pool(name="pp", bufs=1, space="PSUM") as pp,
        ):
            # ---- load constants: CB1 first (gates stage1), then k, CB2 ----
            CBs = [cp.tile(list(b.shape), BF16, tag=f"CB{i}", name=f"CB{i}")
                   for i, b in enumerate(blobs)]
            nc.sync.dma_start(out=CBs[0][:], in_=cb_ins[0][:])
            fsbs = []
            for b in range(B_PER_CORE):
                fsb = cp.tile([64, 512], BF16, tag=f"fsb{b}", name="fsb")
                nc.sync.dma_start(
                    out=fsb[:].rearrange("x (i y) -> x i y", i=8),
                    in_=f_in[b].rearrange("i x y -> x i y"))
                fsbs.append(fsb)
            k_sb = cp.tile([128, 2048], BF16, tag="k_sb", name="k_sb")
            nc.sync.dma_start(out=k_sb[:], in_=k_in[:])
            nc.sync.dma_start(out=CBs[1][:], in_=cb_ins[1][:])

            def cview(name):
                bi, r, w, o = lay[name]
                return CBs[bi][0:r, o:o + w]

            cExFT1 = cview("ExFT1")
            cEyCT = cview("EyCT")
            cEyST = cview("EyST")
            cS_sel = cview("S_sel")
            ctsg = cview("tsg")
            cQF1 = cview("QF1")
            cQF2 = cview("QF2")
            cPRT = cview("PRT")
            cnPIT = cview("nPIT")
            cCyT = cview("CyT")

            # global greedy balancer across DVE / ACT / Pool
    
            es = EngSched({'v': nc.vector, 'a': nc.scalar, 'p': nc.gpsimd})

            def cp_copy(out, in_, free):
                # PSUM->SBUF copy; cost model: ACT 0.833/el + bubble,
                # DVE 1.042/el + bubble, Pool 0.833/el (no errata bubble)
                eng = es.pick({'a': free * 0.833 + 190,
                               'v': free * 1.042 + 180,
                               'p': free * 0.833 + 80})
                if eng is nc.scalar:
                    eng.copy(out=out, in_=in_)
                else:
                    eng.tensor_copy(out, in_)

            st = {b: {} for b in range(B_PER_CORE)}

            def stage1(b):
                s = st[b]
                s['OUT'] = fld.tile([128, NCH_OUT * 128], F32, tag="OUT",
                                    name="OUT")
                fsb = fsbs[b]
                A_sb = []
                for ip in range(4):
                    psA = pp.tile([128, 320], F32, tag="bankA", bufs=2, name="psA")
                    nc.tensor.matmul(psA[:], fsb[:, ip * 128:(ip + 1) * 128],
                                     cExFT1, start=True, stop=True)
                    a_t = wk.tile([128, 320], BF16, tag=f"a{ip}", name="a_t",
                                  bufs=1)
                    cp_copy(a_t[:], psA[:], 320)
                    A_sb.append(a_t)
                s['A_sb'] = A_sb

            def stage_fr(b):
                s = st[b]
                A_sb, OUT = s['A_sb'], s['OUT']
                for iph in range(4):   # 2 channels per psUf tile
                    psUf = pp.tile([128, 256], F32, tag="bankA", bufs=2, name="psUf")
                    for iloc2 in range(2):
                        i = 2 * iph + iloc2
                        ip, iloc = i // 2, i % 2
                        t1 = A_sb[ip][iloc * 64:(iloc + 1) * 64, 192:320]
                        nc.tensor.matmul(psUf[:, iloc2 * 128:(iloc2 + 1) * 128],
                                         t1, cCyT[iloc * 64:(iloc + 1) * 64, :],
                                         start=True, stop=True)
                    cp_copy(OUT[:, iph * 256:(iph + 1) * 256], psUf[:], 256)
                if 'dma' not in ablate:
                    nc.sync.dma_start(
                        out=out_sh[b, :, 0:8, :],
                        in_=OUT[:, 0:1024].rearrange("x (c y) -> x c y", c=8))

            def stage2(b):
                s = st[b]
                A_sb = s['A_sb']
                psFcv = [pp.tile([128, 128], F32, tag=f"bankF{4 + h}", name="psFcv")
                         for h in range(2)]
                for i in range(8):
                    iloc = i % 2
                    ysl = slice(iloc * 64, (iloc + 1) * 64)
                    A_RI = A_sb[i // 2][ysl, 0:128]
                    A_IS = A_sb[i // 2][ysl, 64:192]
                    h, im = i // 4, i % 4
                    sl = slice(im * 32, (im + 1) * 32)
                    tp = (0, im * 32)
                    nc.tensor.matmul(psFcv[h][sl, :], cEyCT[ysl, :], A_RI,
                                     start=True, stop=False, tile_position=tp)
                    nc.tensor.matmul(psFcv[h][sl, :], cEyST[ysl, :], A_IS,
                                     start=False, stop=True, tile_position=tp)
                Fcv = wk.tile([128, 256], BF16, tag="Fcv", name="Fcv")
                for h in range(2):
                    cp_copy(_view(Fcv[:], h * 64,
                                  [Fcv[:].ap[0], [128, 2], [1, 64]]),
                            psFcv[h][:].rearrange("p (r k) -> p r k", r=2), 128)
                s['Fcv'] = Fcv

            def stage_conv(b):
                s = st[b]
                Fcv = s['Fcv']
                Mw = []
                for RI in range(2):
                    m_t = mwp.tile([128, 2048], BF16, tag=f"mw{RI}", name="m_t")
                    conv_eng = es.pick({'v': 2048 * 0.52 + 60,
                                        'p': 2048 * 0.833 + 60})
                    conv_eng.tensor_mul(
                        m_t[:].rearrange("p (j f) -> p j f", j=16),
                        _bcast(Fcv[:, RI * 128:(RI + 1) * 128], 16),
                        k_sb[:].rearrange("p (j f) -> p j f", j=16))
                    Mw.append(m_t)

                acv_sb = wk.tile([32, 2048], BF16, tag="acv", name="acv_sb",
                                 bufs=1)
                for RI in range(2):
                    for jh in range(2):
                        ps_acv = pp.tile([32, 512], F32, tag="bankA", bufs=2,
                                         name="ps_acv")
                        for h in range(2):
                            rhs = _view(Mw[RI][:], jh * 1024 + h * 64,
                                        [Mw[RI][:].ap[0], [128, 8], [1, 64]])
                            nc.tensor.matmul(ps_acv[:], cS_sel, rhs,
                                             start=(h == 0), stop=(h == 1))
                        cp_copy(
                            acv_sb[:, (RI * 2 + jh) * 512:(RI * 2 + jh + 1) * 512],
                            ps_acv[:], 512)
                s['acv_sb'] = acv_sb

            def stage_B(b):
                s = st[b]
                acv_sb = s['acv_sb']
                # ---------------- uncurl: B = acv (*) t/s ----------------
                BuR = wk.tile([32, 1024], BF16, tag="BuR", name="BuR")
                BuI = wk.tile([32, 1024], BF16, tag="BuI", name="BuI")
                BvR = wk.tile([32, 1024], BF16, tag="BvR", name="BvR")
                BvI = wk.tile([32, 1024], BF16, tag="BvI", name="BvI")
                # acv layout: [R jh0 | R jh1 | I jh0 | I jh1] each 512
                # tsg: [-t | t | -s | s] each 512 (j-repeated, j-independent)
                for RI in range(2):
                    a_v = _view(acv_sb[:], RI * 1024,
                                [acv_sb[:].ap[0], [512, 2], [1, 512]])
                    bc = {'v': 1024 * 0.52 + 60, 'p': 1024 * 0.833 + 60}
                    if RI == 0:  # A_R -> imag parts (mult by +t / +s)
                        es.pick(bc).tensor_mul(
                            BuI[:].rearrange("p (j f) -> p j f", j=2),
                            a_v, _bcast(ctsg[:, 512:1024], 2))
                        es.pick(bc).tensor_mul(
                            BvI[:].rearrange("p (j f) -> p j f", j=2),
                            a_v, _bcast(ctsg[:, 1536:2048], 2))
                    else:        # A_I -> real parts (mult by -t / -s)
                        es.pick(bc).tensor_mul(
                            BuR[:].rearrange("p (j f) -> p j f", j=2),
                            a_v, _bcast(ctsg[:, 0:512], 2))
                        es.pick(bc).tensor_mul(
                            BvR[:].rearrange("p (j f) -> p j f", j=2),
                            a_v, _bcast(ctsg[:, 1024:1536], 2))
                s['B'] = (BuR, BuI, BvR, BvI)
                s['u_all'] = fld.tile([128, 2048], BF16, tag="u_all", name="u_all")
                s['v_all'] = fld.tile([128, 2048], BF16, tag="v_all", name="v_all")

            tog_ctr = [0]

            def synth_group(b, field, g2):
                """4 channels (2 cpairs) -> dest[:, g2*512:(g2+1)*512]."""
                s = st[b]
                BuR, BuI, BvR, BvI = s['B']
                BR, BI = (BuR, BuI) if field == 'u' else (BvR, BvI)
                dest = s['u_all'] if field == 'u' else s['v_all']
                tog = tog_ctr[0]
                tog_ctr[0] += 1
                psG = pp.tile([128, 512], F32, tag=f"bankF{tog % 2}",
                              name="psG")
                for sub in range(2):
                    cpair = 2 * g2 + sub
                    csl = slice(cpair * 128, (cpair + 1) * 128)
                    osl = slice(sub * 256, (sub + 1) * 256)
                    nc.tensor.matmul(psG[:, osl], BR[:, csl], cQF1,
                                     start=True, stop=False)
                    nc.tensor.matmul(psG[:, osl], BI[:, csl], cQF2,
                                     start=False, stop=True)
                G_sb = wk.tile([128, 512], BF16, tag="G_sb", name="G_sb")
                cp_copy(G_sb[:], psG[:], 512)
                psU = pp.tile([128, 512], F32, tag=f"bankF{2 + tog % 2}",
                              name="psU")
                for chl in range(4):
                    sub, chp = chl // 2, chl % 2
                    gr = G_sb[chp * 64:(chp + 1) * 64,
                              sub * 256:sub * 256 + 128]
                    gi = G_sb[chp * 64:(chp + 1) * 64,
                              sub * 256 + 128:sub * 256 + 256]
                    psl = slice(chp * 64, (chp + 1) * 64)
                    osl = slice(chl * 128, (chl + 1) * 128)
                    nc.tensor.matmul(psU[:, osl], cPRT[psl, :], gr,
                                     start=True, stop=False)
                    nc.tensor.matmul(psU[:, osl], cnPIT[psl, :], gi,
                                     start=False, stop=True)
                cp_copy(dest[:, g2 * 512:(g2 + 1) * 512], psU[:], 512)

            def emit_cross_block(b, gI, gJ):
                u_all, v_all, OUT = st[b]['u_all'], st[b]['v_all'], st[b]['OUT']
                W1 = wpp.tile([128, 2048], BF16, tag="W1", name="W1")
                # one 4D-AP product op per W tile: [p, a(4), b(4), 128]
                in0 = _view(u_all[:], gI * 512,
                            [u_all[:].ap[0], [128, 4], [0, 4], [1, 128]])
                in1 = _view(v_all[:], gJ * 512,
                            [v_all[:].ap[0], [0, 4], [128, 4], [1, 128]])
                es.pick({'v': 2048 * 0.52 + 60,
                         'p': 2048 * 0.833 + 60}).tensor_mul(
                    W1[:].rearrange("p (a cb f) -> p a cb f", a=4, cb=4),
                    in0, in1)
                if gI != gJ:
                    W2 = wpp.tile([128, 2048], BF16, tag="W2", name="W2")
                    in0 = _view(u_all[:], gJ * 512,
                                [u_all[:].ap[0], [128, 4], [0, 4], [1, 128]])
                    in1 = _view(v_all[:], gI * 512,
                                [v_all[:].ap[0], [0, 4], [128, 4], [1, 128]])
                    es.pick({'v': 2048 * 0.52 + 60,
                             'p': 2048 * 0.833 + 60}).tensor_mul(
                        W2[:].rearrange("p (bj ca f) -> p bj ca f", bj=4, ca=4),
                        in0, in1)
                    for ai in range(4):
                        a = 4 * gI + ai
                        pch = 8 + _PAIR_IDX[(a, 4 * gJ)]
                        in0 = W1[:, ai * 512:(ai + 1) * 512].rearrange(
                            "p (cb f) -> p cb f", cb=4)
                        in1 = _view(W2[:], ai * 128,
                                    [W2[:].ap[0], [512, 4], [1, 128]])
                        out = _view(OUT[:], pch * 128,
                                    [OUT[:].ap[0], [128, 4], [1, 128]])
                        es.pick({'v': 512 * 1.042 + 60,
                                 'p': 512 * 0.833 + 60}).tensor_sub(out, in0, in1)
                else:
                    for ai in range(3):
                        a = 4 * gI + ai
                        cnt = 3 - ai
                        pch = 8 + _PAIR_IDX[(a, a + 1)]
                        in0 = _view(W1[:], ai * 512 + (ai + 1) * 128,
                                    [W1[:].ap[0], [128, cnt], [1, 128]])
                        in1 = _view(W1[:], (ai + 1) * 512 + ai * 128,
                                    [W1[:].ap[0], [512, cnt], [1, 128]])
                        out = _view(OUT[:], pch * 128,
                                    [OUT[:].ap[0], [128, cnt], [1, 128]])
                        es.pick({'v': cnt * 128 * 1.042 + 60,
                                 'p': cnt * 128 * 0.833 + 60}).tensor_sub(
                            out, in0, in1)

            def cross_dma(b, c0, c1):
                OUT = st[b]['OUT']
                nc.sync.dma_start(
                    out=out_sh[b, :, c0:c1, :],
                    in_=OUT[:, c0 * 128:c1 * 128].rearrange(
                        "x (c y) -> x c y", c=c1 - c0))

            # ---------------- staggered emission across samples ----------
            # Rows processed in reverse (small rows first) so the out-DMA
            # stream starts early; b1 runs ~half a pipeline behind b0 so its
            # PE/ACT front work overlaps b0's DVE/Pool cross work.
            chunk_rng = {0: (8, 62), 1: (62, 100), 2: (100, 122), 3: (122, 128)}

            def front(b):
                stage1(b)
                stage_fr(b)
                stage2(b)
                stage_conv(b)
                stage_B(b)

            def row(b, gI):
                synth_group(b, 'u', gI)
                for gJ in range(gI, 4):
                    emit_cross_block(b, gI, gJ)
                if 'dma' not in ablate:
                    cross_dma(b, *chunk_rng[gI])

            front(0)
            synth_group(0, 'v', 3)
            row(0, 3)
            front(1)
            synth_group(0, 'v', 2)
            row(0, 2)
            synth_group(1, 'v', 3)
            row(1, 3)
            synth_group(0, 'v', 1)
            row(0, 1)
            synth_group(1, 'v', 2)
            row(1, 2)
            synth_group(0, 'v', 0)
            row(0, 0)
            synth_group(1, 'v', 1)
            row(1, 1)
            synth_group(1, 'v', 0)
            row(1, 0)
    nc.compile()
    return nc


# ---------------------------------------------------------------------------
# entry point
# ---------------------------------------------------------------------------

_PROGRAM = {}


def _get_program(reps=1, ablate=(), **kw):
    key = (reps, tuple(sorted(ablate)), tuple(sorted(kw.items())))
    if key not in _PROGRAM:
        _PROGRAM[key] = build_program(reps, ablate=ablate, **kw)
    return _PROGRAM[key]


LAST_EXEC_NS = None
LAST_RESULT = None


def kernel(f, kernel):
    global LAST_EXEC_NS, LAST_RESULT
    f_bf = np.ascontiguousarray(np.asarray(f), dtype=np.float32).astype(NPBF16)
    k_all = _prep_k_all(np.asarray(kernel))
    blobs, _ = _host_consts()
    nc = _get_program()
    in_maps = [
        {"f_in": f_bf[2 * c:2 * c + 2], "k_all": k_all,
         "cb0": blobs[0], "cb1": blobs[1]}
        for c in range(N_CORES)
    ]
    import os
    trace = bool(os.environ.get("KERNEL_TRACE"))
    res = run_bass_kernel_spmd(nc, in_maps, list(range(N_CORES)), trace=trace)
    LAST_RESULT = res
    if res.exec_time_ns is not None:
        LAST_EXEC_NS = res.exec_time_ns
    out = np.concatenate([res.results[c]["out_sh"] for c in range(N_CORES)], axis=0)
    # device layout is [b, X, ch, Y]; return the [b, ch, X, Y] view
    return out.transpose(0, 2, 1, 3)
